# revision 1
# baseline (speedup 1.0000x reference)
"""CoNHD GD-layer Trainium2 kernel (8-core SPMD, Bass/Tile).

Math (see module docstring of the reference): two independent set-attention
stacks over fixed-size mailbox groups (v-side: N=2048 nodes x DV=32, e-side:
M=4096 hyperedges x DE=16), followed by a 4*D -> D update linear applied in
two eid orders.

Device strategy:
  - Shard rows (E=65536) across 8 cores: core c owns v-rows and e-rows
    [c*8192, (c+1)*8192).  Group attention never crosses that boundary.
  - Transposed activation layout on chip: SBUF tiles are [feat, rows].
  - All matmuls in fp32r (full PE rate at moving-dim 256, ~1e-4 rel err).
  - Block-diagonal group masking is folded into the score matmul as a
    rank-G accumulation (Gk^T @ Gq, scaled 16C), removed by exp(x/16 - C).
  - Softmax denominator comes from a ones-column appended to V (65-stride
    layout); normalization uses a K=1 ones-matmul broadcast of 1/denom.
  - The update linear is decomposed by column blocks of upd_W:
      A  = co_in@W1 + co_v@W2 + co_0@W4 + b   ('in' eid order, on device)
      P3 = co_e@W3                            ('con' eid order, on device)
      out_in  = A + P3[inv_perm]              (host add + gather)
      out_con = out_in[perm]                  (host gather)

kernel(**inputs) takes the full unsharded inputs and returns [2, E, D] f32.
"""
import sys

if "/opt/trn_rl_repo" not in sys.path:
    sys.path.insert(0, "/opt/trn_rl_repo")

from contextlib import ExitStack

import numpy as np

import concourse.mybir as mybir
import concourse.tile as tile
from concourse import bacc
from concourse.bass_utils import run_bass_kernel_spmd

F32 = mybir.dt.float32
F32R = mybir.dt.float32r
AF = mybir.ActivationFunctionType

N, DV, M, DE, E = 2048, 32, 4096, 16, 65536
D, WD, L, H = 256, 64, 2, 4
NCORES = 8
MASK_C = 30.0


def _sab_tile(nc, pools, Xt, W, Bcol, bvbc, Gk, Gq, ones1, negc):
    """One SAB layer on one 256-row tile (Xt = [feat,rows] tile pair)."""
    sb, psum_mm, psum_S, psum_O = pools

    Qt = [sb.tile([128, 256], F32R, tag="Qt", name="Qt") for _ in range(2)]
    Kt = [sb.tile([128, 256], F32R, tag="Kt", name="Kt") for _ in range(2)]
    for fb in range(2):
        psQ = psum_mm.tile([128, 256], F32, tag="mm", name="psQ")
        for kb in range(2):
            nc.tensor.matmul(psQ[:], W["q"][kb][:, fb * 128:(fb + 1) * 128],
                             Xt[kb][:], start=(kb == 0), stop=(kb == 1))
        nc.vector.tensor_scalar_add(Qt[fb][:], psQ[:], Bcol[:, 0 * 2 + fb:0 * 2 + fb + 1])
        psK = psum_mm.tile([128, 256], F32, tag="mm", name="psK")
        for kb in range(2):
            nc.tensor.matmul(psK[:], W["k"][kb][:, fb * 128:(fb + 1) * 128],
                             Xt[kb][:], start=(kb == 0), stop=(kb == 1))
        nc.vector.tensor_scalar_add(Kt[fb][:], psK[:], Bcol[:, 1 * 2 + fb:1 * 2 + fb + 1])

    # V in row-major 65-stride layout; col h*65+64 holds ones -> softmax denom
    V65 = []
    for rb in range(2):
        psV = psum_mm.tile([128, 256], F32, tag="mm", name="psV")
        for kb in range(2):
            nc.tensor.matmul(psV[:], Xt[kb][:, rb * 128:(rb + 1) * 128],
                             W["v"][kb][:], start=(kb == 0), stop=(kb == 1))
        v65 = sb.tile([128, 4 * 65], F32R, tag="V65", name="V65")
        for h in range(H):
            nc.vector.tensor_add(v65[:, h * 65:h * 65 + 64],
                                 psV[:, h * 64:(h + 1) * 64],
                                 bvbc[:, h * 64:(h + 1) * 64])
        nc.vector.tensor_copy(v65[:, 64::65], ones1[:, 0:4])
        V65.append(v65)

    # attention per head-pair (2 psO banks live at a time)
    Ot = [sb.tile([128, 256], F32R, tag="Ot", name="Ot") for _ in range(2)]
    for pair in range(2):
        psO, recips = [], []
        for hh in range(2):
            h = pair * 2 + hh
            off = hh * 64
            Qht = Qt[pair][off:off + 64, :]
            Kht = Kt[pair][off:off + 64, :]
            eS = []
            for b in range(2):
                psS = psum_S.tile([128, 256], F32, tag="psS", name="psS")
                nc.tensor.matmul(psS[:], Kht[:, b * 128:(b + 1) * 128], Qht,
                                 start=True, stop=False)
                nc.tensor.matmul(psS[:], Gk[:], Gq[b][:], start=False, stop=True)
                e = sb.tile([128, 256], F32R, tag="eS", name="eS")
                nc.scalar.activation(e[:], psS[:], AF.Exp, bias=negc[:], scale=1.0 / 16.0)
                eS.append(e)
            pO = psum_O.tile([65, 256], F32, tag="psO", name="psO")
            for b in range(2):
                nc.tensor.matmul(pO[:], V65[b][:, h * 65:h * 65 + 65], eS[b][:],
                                 start=(b == 0), stop=(b == 1))
            rec = sb.tile([1, 256], F32R, tag="recipH", name="recipH")
            nc.vector.reciprocal(rec[:], pO[64:65, :])
            psO.append(pO)
            recips.append(rec)
        RB = sb.tile([128, 256], F32, tag="RB", name="RB")
        for hh in range(2):
            psRB = psum_S.tile([64, 256], F32, tag="psS", name="psRB")
            nc.tensor.matmul(psRB[:], ones1[0:1, 0:64], recips[hh][:],
                             start=True, stop=True)
            nc.scalar.copy(RB[hh * 64:(hh + 1) * 64, :], psRB[:])
        for hh in range(2):
            off = hh * 64
            nc.vector.tensor_mul(Ot[pair][off:off + 64, :], psO[hh][0:64, :],
                                 RB[off:off + 64, :])
            nc.vector.tensor_add(Ot[pair][off:off + 64, :], Ot[pair][off:off + 64, :],
                                 Qt[pair][off:off + 64, :])

    # Z = O + relu(O @ Wo + bo)
    Zt = [sb.tile([128, 256], F32R, tag="Zt", name="Zt") for _ in range(2)]
    for fb in range(2):
        psR = psum_mm.tile([128, 256], F32, tag="mm", name="psR")
        for kb in range(2):
            nc.tensor.matmul(psR[:], W["o"][kb][:, fb * 128:(fb + 1) * 128],
                             Ot[kb][:], start=(kb == 0), stop=(kb == 1))
        Rt = sb.tile([128, 256], F32, tag="Rt", name="Rt")
        nc.scalar.activation(Rt[:], psR[:], AF.Relu,
                             bias=Bcol[:, 3 * 2 + fb:3 * 2 + fb + 1])
        nc.vector.tensor_add(Zt[fb][:], Ot[fb][:], Rt[:])
    return Zt


def _load_side_consts(nc, const, tag, W_d, Bcol_d, bvbc_d, Gk_d, Gq_d, G):
    Ws, Bcols, bvbcs = [], [], []
    for l in range(L):
        Wl = {}
        for pi, p in enumerate(["q", "k", "v", "o"]):
            Wl[p] = []
            for kb in range(2):
                t = const.tile([128, 256], F32R, tag=f"{tag}W{l}{p}{kb}",
                               name=f"{tag}W{l}{p}{kb}")
                nc.sync.dma_start(t[:], W_d[l, pi, kb * 128:(kb + 1) * 128, :])
                Wl[p].append(t)
        Ws.append(Wl)
        bc = const.tile([128, 8], F32, tag=f"{tag}Bcol{l}", name=f"{tag}Bcol{l}")
        nc.sync.dma_start(bc[:], Bcol_d[l])
        Bcols.append(bc)
        bv = const.tile([128, 256], F32, tag=f"{tag}bvbc{l}", name=f"{tag}bvbc{l}")
        nc.sync.dma_start(bv[:], bvbc_d[l])
        bvbcs.append(bv)
    Gk = const.tile([G, 128], F32R, tag=f"{tag}Gk", name=f"{tag}Gk")
    nc.sync.dma_start(Gk[:], Gk_d)
    Gq = []
    for b in range(2):
        g = const.tile([G, 256], F32R, tag=f"{tag}Gq{b}", name=f"{tag}Gq{b}")
        nc.sync.dma_start(g[:], Gq_d[b])
        Gq.append(g)
    return Ws, Bcols, bvbcs, Gk, Gq


def build_program(R):
    """Build the per-core SPMD program; R = rows per core (multiple of 256)."""
    NT = R // 256
    nc = bacc.Bacc("TRN2", target_bir_lowering=False, debug=False)

    dram = {}

    def din(name, shape, dt=F32R):
        dram[name] = nc.dram_tensor(name, shape, dt, kind="ExternalInput").ap()
        return dram[name]

    xvt_d = din("xvt", [D, R])
    wvt_d = din("wvt", [WD, R])
    xet_d = din("xet", [D, R])
    wet_d = din("wet", [WD, R])
    x0t_d = din("x0t", [D, R])
    peW_v_d = din("peW_v", [WD, D])
    peW_e_d = din("peW_e", [WD, D])
    peb_v_d = din("peb_v", [D], F32)
    peb_e_d = din("peb_e", [D], F32)
    Wv_d = din("W_v", [L, 4, D, D])
    We_d = din("W_e", [L, 4, D, D])
    Bcol_v_d = din("Bcol_v", [L, 128, 8], F32)
    Bcol_e_d = din("Bcol_e", [L, 128, 8], F32)
    bvbc_v_d = din("bvbc_v", [L, 128, D], F32)
    bvbc_e_d = din("bvbc_e", [L, 128, D], F32)
    Wupd_d = din("W_upd", [4, D, D])
    updb_d = din("updb_bc", [128, D], F32)
    Gk_v_d = din("Gk_v", [4, 128])
    Gq_v_d = din("Gq_v", [2, 4, 256])
    Gk_e_d = din("Gk_e", [8, 128])
    Gq_e_d = din("Gq_e", [2, 8, 256])
    ones1_d = din("ones1", [128, 128])

    A_d = nc.dram_tensor("A", [R, D], F32, kind="ExternalOutput").ap()
    P3_d = nc.dram_tensor("P3", [R, D], F32, kind="ExternalOutput").ap()

    with tile.TileContext(nc) as tc, ExitStack() as es, \
            nc.allow_low_precision(reason="fp32r matmul pipeline, fp32 accum in PSUM"):
        const = es.enter_context(tc.tile_pool(name="const", bufs=1))
        sb = es.enter_context(tc.tile_pool(name="sb", bufs=4))
        inp = es.enter_context(tc.tile_pool(name="inp", bufs=4))
        outp = es.enter_context(tc.tile_pool(name="outp", bufs=4))
        psum_mm = es.enter_context(tc.tile_pool(name="psmm", bufs=3, space="PSUM"))
        psum_S = es.enter_context(tc.tile_pool(name="psS", bufs=3, space="PSUM"))
        psum_O = es.enter_context(tc.tile_pool(name="psO", bufs=2, space="PSUM"))
        pools = (sb, psum_mm, psum_S, psum_O)

        negc = const.tile([128, 1], F32, tag="negc", name="negc")
        nc.vector.memset(negc[:], -MASK_C)
        ones1 = const.tile([128, 128], F32R, tag="ones1", name="ones1")
        nc.sync.dma_start(ones1[:], ones1_d)

        peW = {}
        peb = {}
        for s, peW_d, peb_d in (("v", peW_v_d, peb_v_d), ("e", peW_e_d, peb_e_d)):
            t = const.tile([WD, D], F32R, tag=f"peW_{s}", name=f"peW_{s}")
            nc.sync.dma_start(t[:], peW_d)
            peW[s] = t
            b = const.tile([128, 2], F32, tag=f"peb_{s}", name=f"peb_{s}")
            for fb in range(2):
                nc.sync.dma_start(b[:, fb:fb + 1],
                                  peb_d[fb * 128:(fb + 1) * 128].unsqueeze(-1))
            peb[s] = b

        side_consts = {
            "v": _load_side_consts(nc, const, "v", Wv_d, Bcol_v_d, bvbc_v_d,
                                   Gk_v_d, Gq_v_d, 4),
            "e": _load_side_consts(nc, const, "e", We_d, Bcol_e_d, bvbc_e_d,
                                   Gk_e_d, Gq_e_d, 8),
        }

        Wupd = []
        for j in range(4):
            Wupd.append([])
            for kb in range(2):
                t = const.tile([128, 256], F32R, tag=f"Wupd{j}{kb}", name=f"Wupd{j}{kb}")
                nc.sync.dma_start(t[:], Wupd_d[j, kb * 128:(kb + 1) * 128, :])
                Wupd[j].append(t)
        updb = const.tile([128, 256], F32, tag="updb", name="updb")
        nc.sync.dma_start(updb[:], updb_d)

        for side in ("v", "e"):
            Ws, Bcols, bvbcs, Gk, Gq = side_consts[side]
            xt_d, wt_d = (xvt_d, wvt_d) if side == "v" else (xet_d, wet_d)
            for t in range(NT):
                cs = slice(t * 256, (t + 1) * 256)
                xt = [inp.tile([128, 256], F32R, tag=f"xt{side}", name="xt")
                      for _ in range(2)]
                for fb in range(2):
                    nc.sync.dma_start(xt[fb][:], xt_d[fb * 128:(fb + 1) * 128, cs])
                wt = inp.tile([WD, 256], F32R, tag=f"wt{side}", name="wt")
                nc.sync.dma_start(wt[:], wt_d[:, cs])

                # mailbox: Xt = xt + peW^T wt + peb
                Xt = [sb.tile([128, 256], F32R, tag="Xt", name="Xt") for _ in range(2)]
                for fb in range(2):
                    psP = psum_mm.tile([128, 256], F32, tag="mm", name="psP")
                    nc.tensor.matmul(psP[:], peW[side][:, fb * 128:(fb + 1) * 128],
                                     wt[:], start=True, stop=True)
                    nc.vector.scalar_tensor_tensor(
                        Xt[fb][:], psP[:], peb[side][:, fb:fb + 1], xt[fb][:],
                        mybir.AluOpType.add, mybir.AluOpType.add)

                for l in range(L):
                    Xt = _sab_tile(nc, pools, Xt, Ws[l], Bcols[l], bvbcs[l],
                                   Gk, Gq, ones1, negc)

                if side == "v":
                    x0 = [inp.tile([128, 256], F32R, tag="x0", name="x0")
                          for _ in range(2)]
                    for fb in range(2):
                        nc.sync.dma_start(x0[fb][:], x0t_d[fb * 128:(fb + 1) * 128, cs])
                    for rb in range(2):
                        rs = slice(rb * 128, (rb + 1) * 128)
                        psA = psum_mm.tile([128, 256], F32, tag="mm", name="psA")
                        first = True
                        for src, j in ((xt, 0), (Xt, 1), (x0, 3)):
                            for kb in range(2):
                                nc.tensor.matmul(psA[:], src[kb][:, rs], Wupd[j][kb][:],
                                                 start=first,
                                                 stop=(src is x0 and kb == 1))
                                first = False
                        Ao = outp.tile([128, 256], F32, tag="Aout", name="Aout")
                        nc.vector.tensor_add(Ao[:], psA[:], updb[:])
                        nc.sync.dma_start(A_d[t * 256 + rb * 128:t * 256 + (rb + 1) * 128, :],
                                          Ao[:])
                else:
                    for rb in range(2):
                        rs = slice(rb * 128, (rb + 1) * 128)
                        psP3 = psum_mm.tile([128, 256], F32, tag="mm", name="psP3")
                        for kb in range(2):
                            nc.tensor.matmul(psP3[:], Xt[kb][:, rs], Wupd[2][kb][:],
                                             start=(kb == 0), stop=(kb == 1))
                        Po = outp.tile([128, 256], F32, tag="Pout", name="Pout")
                        nc.vector.tensor_copy(Po[:], psP3[:])
                        nc.sync.dma_start(P3_d[t * 256 + rb * 128:t * 256 + (rb + 1) * 128, :],
                                          Po[:])

    nc.compile()
    return nc


def _make_group_consts(n_group):
    G = 128 // n_group
    Gk = np.zeros((G, 128), np.float32)
    for g in range(G):
        Gk[g, g * n_group:(g + 1) * n_group] = 16.0 * MASK_C
    Gq = np.zeros((2, G, 256), np.float32)
    for b in range(2):
        for g in range(G):
            q0 = b * 128 + g * n_group
            Gq[b, g, q0:q0 + n_group] = 1.0
    return Gk, Gq


def _pack_bcol(bq, bk, bv, bo):
    """[128, 8] bias columns per layer: col p*2+fb."""
    out = np.zeros((L, 128, 8), np.float32)
    for l in range(L):
        for pi, b in enumerate((bq, bk, bv, bo)):
            for fb in range(2):
                out[l, :, pi * 2 + fb] = b[l, fb * 128:(fb + 1) * 128]
    return out


_PROGRAM_CACHE = {}


def _get_program(R):
    if R not in _PROGRAM_CACHE:
        _PROGRAM_CACHE[R] = build_program(R)
    return _PROGRAM_CACHE[R]


def kernel(co_feat_in, co_feat_con, co_feat_0, weight_in, weight_con,
           pe_v_W, pe_v_b, pe_e_W, pe_e_b,
           Wq_v, bq_v, Wk_v, bk_v, Wv_v, bv_v, Wo_v, bo_v,
           Wq_e, bq_e, Wk_e, bk_e, Wv_e, bv_e, Wo_e, bo_e,
           upd_W, upd_b, perm):
    f = np.asarray
    co_feat_in = f(co_feat_in, np.float32)
    co_feat_con = f(co_feat_con, np.float32)
    co_feat_0 = f(co_feat_0, np.float32)
    weight_in = f(weight_in, np.float32)
    weight_con = f(weight_con, np.float32)
    perm = np.asarray(perm)

    R = E // NCORES
    nc = _get_program(R)

    Gk_v, Gq_v = _make_group_consts(DV)
    Gk_e, Gq_e = _make_group_consts(DE)

    shared = {
        "peW_v": f(pe_v_W, np.float32), "peW_e": f(pe_e_W, np.float32),
        "peb_v": f(pe_v_b, np.float32), "peb_e": f(pe_e_b, np.float32),
        "W_v": np.stack([f(Wq_v, np.float32), f(Wk_v, np.float32),
                         f(Wv_v, np.float32), f(Wo_v, np.float32)], axis=1).copy(),
        "W_e": np.stack([f(Wq_e, np.float32), f(Wk_e, np.float32),
                         f(Wv_e, np.float32), f(Wo_e, np.float32)], axis=1).copy(),
        "Bcol_v": _pack_bcol(f(bq_v, np.float32), f(bk_v, np.float32),
                             f(bv_v, np.float32), f(bo_v, np.float32)),
        "Bcol_e": _pack_bcol(f(bq_e, np.float32), f(bk_e, np.float32),
                             f(bv_e, np.float32), f(bo_e, np.float32)),
        "bvbc_v": np.ascontiguousarray(
            np.broadcast_to(f(bv_v, np.float32)[:, None, :], (L, 128, D))),
        "bvbc_e": np.ascontiguousarray(
            np.broadcast_to(f(bv_e, np.float32)[:, None, :], (L, 128, D))),
        "W_upd": np.ascontiguousarray(
            f(upd_W, np.float32).reshape(4, D, D)),
        "updb_bc": np.ascontiguousarray(
            np.broadcast_to(f(upd_b, np.float32)[None, :], (128, D))),
        "Gk_v": Gk_v, "Gq_v": Gq_v, "Gk_e": Gk_e, "Gq_e": Gq_e,
        "ones1": np.ones((128, 128), np.float32),
    }

    in_maps = []
    for c in range(NCORES):
        rs = slice(c * R, (c + 1) * R)
        m = dict(shared)
        m["xvt"] = np.ascontiguousarray(co_feat_in[rs].T)
        m["wvt"] = np.ascontiguousarray(weight_in[rs].T)
        m["xet"] = np.ascontiguousarray(co_feat_con[rs].T)
        m["wet"] = np.ascontiguousarray(weight_con[rs].T)
        m["x0t"] = np.ascontiguousarray(co_feat_0[rs].T)
        in_maps.append(m)

    global _last_in_maps
    _last_in_maps = in_maps
    res = run_bass_kernel_spmd(nc, in_maps, core_ids=list(range(NCORES)))
    A = np.concatenate([res.results[c]["A"] for c in range(NCORES)], axis=0)
    P3 = np.concatenate([res.results[c]["P3"] for c in range(NCORES)], axis=0)

    inv_perm = np.argsort(perm)
    out_in = A + P3[inv_perm]
    return np.stack([out_in, out_in[perm]]).astype(np.float32)



# revision 2
# speedup vs baseline: 1.1054x; 1.1054x over previous
"""CoNHD GD-layer Trainium2 kernel (8-core SPMD, Bass/Tile), v2.

Math (see the reference): two independent set-attention stacks over
fixed-size mailbox groups (v-side: N=2048 nodes x DV=32, e-side: M=4096
hyperedges x DE=16), followed by a 4*D -> D update linear applied in two
eid orders.

Device strategy (v2 — engine-balanced, block-diagonal attention):
  - Shard rows (E=65536) across 8 cores; group attention never crosses
    the per-core boundary.  Activations are bf16 on chip; PSUM accum fp32.
  - Scores are computed only for the two diagonal 128x128 key x query
    blocks of each 256-row tile (groups of 32/16 never cross them), all
    4 heads sharing one [128,512] PSUM bank; the block-diagonal group
    mask is a rank-G accumulating matmul (exp(x/16 - C) removes 16C).
  - AV is computed transposed (queries on partitions) with a 65-stride
    row-major V whose ones-column yields the softmax denominator, so
    normalization is a cheap per-partition tensor_scalar on DVE.
  - Normalized O is transposed back to feature-major on the PE (identity
    matmul), +Qh and +bv fused in one scalar_tensor_tensor.
  - bk provably cancels in softmax and is dropped; bq/bo/bv/pe_b/upd_b
    ride for free in activation/stt per-partition bias slots.
  - PSUM-evacuation work is spread across Activation / DVE / GPSIMD so
    no single engine bottlenecks; PE is the critical engine.
  - The update linear runs feature-major (out [of, rows]) so upd_b is a
    per-partition bias; outputs land transposed in HBM ([D, R] bf16) and
    the host re-transposes + combines the two eid orders:
      out_in  = A + P3[inv_perm],  out_con = out_in[perm].

kernel(**inputs) takes the full unsharded inputs and returns [2, E, D] f32.
"""
import sys

if "/opt/trn_rl_repo" not in sys.path:
    sys.path.insert(0, "/opt/trn_rl_repo")

from contextlib import ExitStack

import numpy as np

import concourse.mybir as mybir
import concourse.tile as tile
from concourse import bacc
from concourse.bass_utils import run_bass_kernel_spmd

F32 = mybir.dt.float32
BF16 = mybir.dt.bfloat16
AF = mybir.ActivationFunctionType
ALU = mybir.AluOpType

N, DV, M, DE, E = 2048, 32, 4096, 16, 65536
D, WD, L, H = 256, 64, 2, 4
NCORES = 8
MASK_C = 30.0


def _sab_tile(nc, pools, Xt, C, ident, negc):
    """One SAB layer on one 256-row tile.

    Xt: [feat, rows] pair of [128,256] bf16 tiles.  C holds this layer's
    consts: W (q/k/v/o lhsT tiles), Bq/Bo/Bv bias cols, Gk, Gq4.
    Returns the new Xt pair (bf16).
    """
    sb, ps = pools
    W, Bq, Bo, Bv, Gk, Gq4 = C

    # --- Q/K projections (feature-major) ---
    psQ = ps.tile([128, 512], F32, tag="bank", name="psQ")
    psK = ps.tile([128, 512], F32, tag="bank", name="psK")
    for fb in range(2):
        for kb in range(2):
            nc.tensor.matmul(psQ[:, fb * 256:(fb + 1) * 256],
                             W["q"][kb][:, fb * 128:(fb + 1) * 128], Xt[kb][:],
                             start=(kb == 0), stop=(kb == 1))
            nc.tensor.matmul(psK[:, fb * 256:(fb + 1) * 256],
                             W["k"][kb][:, fb * 128:(fb + 1) * 128], Xt[kb][:],
                             start=(kb == 0), stop=(kb == 1))
    Qt = [sb.tile([128, 256], BF16, tag="Qt", name="Qt") for _ in range(2)]
    Kt = [sb.tile([128, 256], BF16, tag="Kt", name="Kt") for _ in range(2)]
    for fb in range(2):
        # bq folded into the evacuation; bk cancels in softmax -> plain copy.
        nc.scalar.activation(Qt[fb][:], psQ[:, fb * 256:(fb + 1) * 256],
                             AF.Identity, bias=Bq[:, fb:fb + 1])
        nc.gpsimd.tensor_copy(Kt[fb][:], psK[:, fb * 256:(fb + 1) * 256])

    # --- V projection (row-major, 65-stride with ones column) ---
    psV = ps.tile([128, 2, 4, 64], F32, tag="bank", name="psV")
    for rb in range(2):
        for kb in range(2):
            nc.tensor.matmul(psV[:, rb, :, :],
                             Xt[kb][:, rb * 128:(rb + 1) * 128], W["v"][kb][:],
                             start=(kb == 0), stop=(kb == 1))
    V65 = sb.tile([128, 2, 4, 65], BF16, tag="V65", name="V65")
    for rb in range(2):
        nc.gpsimd.tensor_copy(V65[:, rb, :, 0:64], psV[:, rb, :, :])
    nc.gpsimd.memset(V65[:, :, :, 64:65], 1.0)

    # --- attention per diagonal 128x128 block ---
    O_rm = sb.tile([128, 2, 256], BF16, tag="Orm", name="O_rm")
    for b in range(2):
        psS = ps.tile([128, 512], F32, tag="bank", name="psS")
        nc.tensor.matmul(psS[:], Gk[:], Gq4[:], start=True, stop=False)
        for h in range(H):
            p, off = h // 2, (h % 2) * 64
            bs = slice(b * 128, (b + 1) * 128)
            nc.tensor.matmul(psS[:, h * 128:(h + 1) * 128],
                             Kt[p][off:off + 64, bs], Qt[p][off:off + 64, bs],
                             start=False, stop=True)
        eS = sb.tile([128, 512], BF16, tag="eS", name="eS")
        nc.scalar.activation(eS[:], psS[:], AF.Exp, bias=negc[:], scale=1.0 / 16.0)

        pO = ps.tile([128, 4, 65], F32, tag="bank", name="pO")
        for h in range(H):
            nc.tensor.matmul(pO[:, h, :], eS[:, h * 128:(h + 1) * 128],
                             V65[:, b, h, :], start=True, stop=True)
        rec = sb.tile([128, 4, 1], F32, tag="rec", name="rec")
        nc.vector.reciprocal(rec[:], pO[:, :, 64:65])
        for h in range(H):
            nc.vector.tensor_scalar_mul(O_rm[:, b, h * 64:(h + 1) * 64],
                                        pO[:, h, 0:64], rec[:, h, 0:1])

    # --- transpose O back to feature-major, + bv + Qh ---
    psOT = ps.tile([128, 2, 256], BF16, tag="bank", name="psOT")
    for b in range(2):
        for fb in range(2):
            nc.tensor.transpose(psOT[:, fb, b * 128:(b + 1) * 128],
                                O_rm[:, b, fb * 128:(fb + 1) * 128], ident[:])
    Ot = [sb.tile([128, 256], BF16, tag="Ot", name="Ot") for _ in range(2)]
    for fb in range(2):
        nc.vector.scalar_tensor_tensor(Ot[fb][:], psOT[:, fb, :],
                                       Bv[:, fb:fb + 1], Qt[fb][:],
                                       ALU.add, ALU.add)

    # --- Z = O + relu(O @ Wo + bo) ---
    psR = ps.tile([128, 512], F32, tag="bank", name="psR")
    for fb in range(2):
        for kb in range(2):
            nc.tensor.matmul(psR[:, fb * 256:(fb + 1) * 256],
                             W["o"][kb][:, fb * 128:(fb + 1) * 128], Ot[kb][:],
                             start=(kb == 0), stop=(kb == 1))
    Rt = [sb.tile([128, 256], BF16, tag="Rt", name="Rt") for _ in range(2)]
    nc.scalar.activation(Rt[0][:], psR[:, 0:256], AF.Relu, bias=Bo[:, 0:1])
    nc.gpsimd.tensor_scalar(Rt[1][:], psR[:, 256:512], Bo[:, 1:2], 0.0,
                            ALU.add, ALU.max)
    Zt = [sb.tile([128, 256], BF16, tag="Zt", name="Zt") for _ in range(2)]
    for fb in range(2):
        nc.vector.tensor_add(Zt[fb][:], Ot[fb][:], Rt[fb][:])
    return Zt


def _load_side_consts(nc, const, tag, W_d, Bq_d, Bo_d, Bv_d, Gk_d, Gq4_d, G):
    layers = []
    for l in range(L):
        Wl = {}
        for pi, p in enumerate(["q", "k", "v", "o"]):
            Wl[p] = []
            for kb in range(2):
                t = const.tile([128, 256], BF16, tag=f"{tag}W{l}{p}{kb}",
                               name=f"{tag}W{l}{p}{kb}")
                nc.sync.dma_start(t[:], W_d[l, pi, kb * 128:(kb + 1) * 128, :])
                Wl[p].append(t)
        bq = const.tile([128, 2], F32, tag=f"{tag}Bq{l}", name=f"{tag}Bq{l}")
        nc.sync.dma_start(bq[:], Bq_d[l])
        bo = const.tile([128, 2], F32, tag=f"{tag}Bo{l}", name=f"{tag}Bo{l}")
        nc.sync.dma_start(bo[:], Bo_d[l])
        bv = const.tile([128, 2], F32, tag=f"{tag}Bv{l}", name=f"{tag}Bv{l}")
        nc.sync.dma_start(bv[:], Bv_d[l])
        layers.append((Wl, bq, bo, bv))
    Gk = const.tile([G, 128], BF16, tag=f"{tag}Gk", name=f"{tag}Gk")
    nc.sync.dma_start(Gk[:], Gk_d)
    Gq4 = const.tile([G, 512], BF16, tag=f"{tag}Gq4", name=f"{tag}Gq4")
    nc.sync.dma_start(Gq4[:], Gq4_d)
    return [(Wl, bq, bo, bv, Gk, Gq4) for (Wl, bq, bo, bv) in layers]


def build_program(R):
    """Build the per-core SPMD program; R = rows per core (multiple of 256)."""
    NT = R // 256
    nc = bacc.Bacc("TRN2", target_bir_lowering=False, debug=False)

    def din(name, shape, dt=BF16):
        return nc.dram_tensor(name, shape, dt, kind="ExternalInput").ap()

    xvt_d = din("xvt", [D, R])
    wvt_d = din("wvt", [WD, R])
    xet_d = din("xet", [D, R])
    wet_d = din("wet", [WD, R])
    x0t_d = din("x0t", [D, R])
    peW_v_d = din("peW_v", [WD, D])
    peW_e_d = din("peW_e", [WD, D])
    peb_v_d = din("peb_v", [128, 2], F32)
    peb_e_d = din("peb_e", [128, 2], F32)
    Wv_d = din("W_v", [L, 4, D, D])
    We_d = din("W_e", [L, 4, D, D])
    Bq_v_d = din("Bq_v", [L, 128, 2], F32)
    Bq_e_d = din("Bq_e", [L, 128, 2], F32)
    Bo_v_d = din("Bo_v", [L, 128, 2], F32)
    Bo_e_d = din("Bo_e", [L, 128, 2], F32)
    Bv_v_d = din("Bv_v", [L, 128, 2], F32)
    Bv_e_d = din("Bv_e", [L, 128, 2], F32)
    Wupd_d = din("W_upd", [4, D, D])
    updb_d = din("updb", [128, 2], F32)
    Gk_v_d = din("Gk_v", [4, 128])
    Gq4_v_d = din("Gq4_v", [4, 512])
    Gk_e_d = din("Gk_e", [8, 128])
    Gq4_e_d = din("Gq4_e", [8, 512])
    ident_d = din("ident", [128, 128])

    At_d = nc.dram_tensor("At", [D, R], BF16, kind="ExternalOutput").ap()
    P3t_d = nc.dram_tensor("P3t", [D, R], BF16, kind="ExternalOutput").ap()

    with tile.TileContext(nc) as tc, ExitStack() as es, \
            nc.allow_low_precision(reason="bf16 activations, fp32 PSUM accum"):
        const = es.enter_context(tc.tile_pool(name="const", bufs=1))
        sb = es.enter_context(tc.tile_pool(name="sb", bufs=3))
        inp = es.enter_context(tc.tile_pool(name="inp", bufs=3))
        outp = es.enter_context(tc.tile_pool(name="outp", bufs=3))
        ps = es.enter_context(tc.tile_pool(name="ps", bufs=7, space="PSUM"))
        pools = (sb, ps)

        negc = const.tile([128, 1], F32, tag="negc", name="negc")
        nc.vector.memset(negc[:], -MASK_C)
        ident = const.tile([128, 128], BF16, tag="ident", name="ident")
        nc.sync.dma_start(ident[:], ident_d)

        peW = {}
        peb = {}
        for s, peW_d, peb_d in (("v", peW_v_d, peb_v_d), ("e", peW_e_d, peb_e_d)):
            t = const.tile([WD, D], BF16, tag=f"peW_{s}", name=f"peW_{s}")
            nc.sync.dma_start(t[:], peW_d)
            peW[s] = t
            b = const.tile([128, 2], F32, tag=f"peb_{s}", name=f"peb_{s}")
            nc.sync.dma_start(b[:], peb_d)
            peb[s] = b

        side_consts = {
            "v": _load_side_consts(nc, const, "v", Wv_d, Bq_v_d, Bo_v_d, Bv_v_d,
                                   Gk_v_d, Gq4_v_d, 4),
            "e": _load_side_consts(nc, const, "e", We_d, Bq_e_d, Bo_e_d, Bv_e_d,
                                   Gk_e_d, Gq4_e_d, 8),
        }

        Wupd = []
        for j in range(4):
            Wupd.append([])
            for kb in range(2):
                t = const.tile([128, 256], BF16, tag=f"Wupd{j}{kb}",
                               name=f"Wupd{j}{kb}")
                nc.sync.dma_start(t[:], Wupd_d[j, kb * 128:(kb + 1) * 128, :])
                Wupd[j].append(t)
        updb = const.tile([128, 2], F32, tag="updb", name="updb")
        nc.sync.dma_start(updb[:], updb_d)

        for side in ("v", "e"):
            consts = side_consts[side]
            xt_d, wt_d = (xvt_d, wvt_d) if side == "v" else (xet_d, wet_d)
            out_d = At_d if side == "v" else P3t_d
            for t in range(NT):
                cs = slice(t * 256, (t + 1) * 256)
                xt = [inp.tile([128, 256], BF16, tag=f"xt{side}", name="xt")
                      for _ in range(2)]
                for fb in range(2):
                    nc.sync.dma_start(xt[fb][:], xt_d[fb * 128:(fb + 1) * 128, cs])
                wt = inp.tile([WD, 256], BF16, tag=f"wt{side}", name="wt")
                nc.sync.dma_start(wt[:], wt_d[:, cs])

                # mailbox: Xt = xt + peW^T wt + peb
                psP = ps.tile([128, 512], F32, tag="bank", name="psP")
                for fb in range(2):
                    nc.tensor.matmul(psP[:, fb * 256:(fb + 1) * 256],
                                     peW[side][:, fb * 128:(fb + 1) * 128],
                                     wt[:], start=True, stop=True)
                Xt = [sb.tile([128, 256], BF16, tag="Xt", name="Xt")
                      for _ in range(2)]
                for fb in range(2):
                    nc.vector.scalar_tensor_tensor(
                        Xt[fb][:], psP[:, fb * 256:(fb + 1) * 256],
                        peb[side][:, fb:fb + 1], xt[fb][:], ALU.add, ALU.add)

                for l in range(L):
                    Xt = _sab_tile(nc, pools, Xt, consts[l], ident, negc)

                # update linear, feature-major: out[of, rows]
                psA = ps.tile([128, 512], F32, tag="bank", name="psA")
                if side == "v":
                    x0 = [inp.tile([128, 256], BF16, tag="x0", name="x0")
                          for _ in range(2)]
                    for fb in range(2):
                        nc.sync.dma_start(x0[fb][:],
                                          x0t_d[fb * 128:(fb + 1) * 128, cs])
                    srcs = ((xt, 0), (Xt, 1), (x0, 3))
                else:
                    srcs = ((Xt, 2),)
                for of in range(2):
                    first = True
                    for src, j in srcs:
                        for kb in range(2):
                            nc.tensor.matmul(
                                psA[:, of * 256:(of + 1) * 256],
                                Wupd[j][kb][:, of * 128:(of + 1) * 128],
                                src[kb][:], start=first,
                                stop=(src is srcs[-1][0] and kb == 1))
                            first = False
                Ao = outp.tile([128, 512], BF16, tag=f"Ao{side}", name="Ao")
                for of in range(2):
                    if side == "v":
                        nc.gpsimd.tensor_scalar_add(
                            Ao[:, of * 256:(of + 1) * 256],
                            psA[:, of * 256:(of + 1) * 256], updb[:, of:of + 1])
                    else:
                        nc.gpsimd.tensor_copy(Ao[:, of * 256:(of + 1) * 256],
                                              psA[:, of * 256:(of + 1) * 256])
                    nc.sync.dma_start(out_d[of * 128:(of + 1) * 128, cs],
                                      Ao[:, of * 256:(of + 1) * 256])

    nc.compile()
    return nc


def _make_group_consts(n_group):
    """Gk [G,128] (bf16, 16C on own group) and Gq4 [G,512] (1 on own group,
    tiled for 4 heads)."""
    G = 128 // n_group
    Gk = np.zeros((G, 128), np.float32)
    Gq = np.zeros((G, 128), np.float32)
    for g in range(G):
        Gk[g, g * n_group:(g + 1) * n_group] = 16.0 * MASK_C
        Gq[g, g * n_group:(g + 1) * n_group] = 1.0
    Gq4 = np.tile(Gq, (1, 4))
    return Gk, Gq4


def _pack_cols(b):
    """[L, D] -> [L, 128, 2] per-partition bias columns per feature chunk."""
    out = np.zeros((L, 128, 2), np.float32)
    for l in range(L):
        for fb in range(2):
            out[l, :, fb] = b[l, fb * 128:(fb + 1) * 128]
    return out


_PROGRAM_CACHE = {}


def _get_program(R):
    if R not in _PROGRAM_CACHE:
        _PROGRAM_CACHE[R] = build_program(R)
    return _PROGRAM_CACHE[R]


def kernel(co_feat_in, co_feat_con, co_feat_0, weight_in, weight_con,
           pe_v_W, pe_v_b, pe_e_W, pe_e_b,
           Wq_v, bq_v, Wk_v, bk_v, Wv_v, bv_v, Wo_v, bo_v,
           Wq_e, bq_e, Wk_e, bk_e, Wv_e, bv_e, Wo_e, bo_e,
           upd_W, upd_b, perm):
    bf = np.dtype(mybir.dt.np(BF16))
    f = lambda x: np.asarray(x, np.float32)
    perm = np.asarray(perm)

    R = E // NCORES
    nc = _get_program(R)

    Gk_v, Gq4_v = _make_group_consts(DV)
    Gk_e, Gq4_e = _make_group_consts(DE)

    def col2(b):
        return np.stack([f(b)[0:128], f(b)[128:256]], axis=1).copy()

    shared = {
        "peW_v": f(pe_v_W).astype(bf), "peW_e": f(pe_e_W).astype(bf),
        "peb_v": col2(pe_v_b), "peb_e": col2(pe_e_b),
        "W_v": np.stack([f(Wq_v), f(Wk_v), f(Wv_v), f(Wo_v)],
                        axis=1).astype(bf).copy(),
        "W_e": np.stack([f(Wq_e), f(Wk_e), f(Wv_e), f(Wo_e)],
                        axis=1).astype(bf).copy(),
        "Bq_v": _pack_cols(f(bq_v)), "Bq_e": _pack_cols(f(bq_e)),
        "Bo_v": _pack_cols(f(bo_v)), "Bo_e": _pack_cols(f(bo_e)),
        "Bv_v": _pack_cols(f(bv_v)), "Bv_e": _pack_cols(f(bv_e)),
        "W_upd": np.ascontiguousarray(f(upd_W).reshape(4, D, D)).astype(bf),
        "updb": col2(upd_b),
        "Gk_v": Gk_v.astype(bf), "Gq4_v": Gq4_v.astype(bf),
        "Gk_e": Gk_e.astype(bf), "Gq4_e": Gq4_e.astype(bf),
        "ident": np.eye(128, dtype=np.float32).astype(bf),
    }

    in_maps = []
    for c in range(NCORES):
        rs = slice(c * R, (c + 1) * R)
        m = dict(shared)
        m["xvt"] = np.asarray(co_feat_in)[rs].T.astype(bf)
        m["wvt"] = np.asarray(weight_in)[rs].T.astype(bf)
        m["xet"] = np.asarray(co_feat_con)[rs].T.astype(bf)
        m["wet"] = np.asarray(weight_con)[rs].T.astype(bf)
        m["x0t"] = np.asarray(co_feat_0)[rs].T.astype(bf)
        in_maps.append(m)

    res = run_bass_kernel_spmd(nc, in_maps, core_ids=list(range(NCORES)))
    A = np.concatenate(
        [res.results[c]["At"].T.astype(np.float32) for c in range(NCORES)], axis=0)
    P3 = np.concatenate(
        [res.results[c]["P3t"].T.astype(np.float32) for c in range(NCORES)], axis=0)

    inv_perm = np.argsort(perm)
    out_in = A + P3[inv_perm]
    return np.stack([out_in, out_in[perm]]).astype(np.float32)


# revision 3
# speedup vs baseline: 1.6922x; 1.5308x over previous
"""CoNHD GD-layer Trainium2 kernel (8-core SPMD, Bass/Tile), v3.

Math (see the reference): two independent set-attention stacks over
fixed-size mailbox groups (v-side: N=2048 nodes x DV=32, e-side: M=4096
hyperedges x DE=16), followed by a 4*D -> D update linear applied in two
eid orders.

Device strategy (engine-balanced, block-diagonal attention, 2-way
software pipelining):
  - Shard rows (E=65536) across 8 cores; group attention never crosses
    the per-core boundary.  Activations are bf16 on chip; PSUM accum fp32.
  - A v-side tile and an e-side tile are processed as a stage-interleaved
    pair: their dependency chains are independent, so each engine's
    in-order queue always has work from the other tile while one tile
    waits on a cross-engine dependency.
  - Scores are computed only for the two diagonal 128x128 key x query
    blocks of each 256-row tile (groups of 32/16 never cross them), all
    4 heads sharing one [128,512] PSUM bank; the block-diagonal group
    mask is a rank-G accumulating matmul (exp(x/16 - C) removes 16C).
  - AV is computed transposed (queries on partitions) with a 65-stride
    row-major V whose ones-column yields the softmax denominator, so
    normalization is a cheap per-partition tensor_scalar.
  - Normalized O is transposed back to feature-major on the PE (identity
    matmul), +Qh and +bv fused in one scalar_tensor_tensor.
  - bk provably cancels in softmax and is dropped; bq/bo/bv/pe_b/upd_b
    ride for free in activation/stt per-partition bias slots.
  - The update linear runs feature-major (out [of, rows]) so upd_b is a
    per-partition bias; outputs land transposed in HBM ([D, R] bf16) and
    the host re-transposes + combines the two eid orders:
      out_in  = A + P3[inv_perm],  out_con = out_in[perm].

kernel(**inputs) takes the full unsharded inputs and returns [2, E, D] f32.
"""
import sys

if "/opt/trn_rl_repo" not in sys.path:
    sys.path.insert(0, "/opt/trn_rl_repo")

from contextlib import ExitStack

import numpy as np

import concourse.mybir as mybir
import concourse.tile as tile
from concourse import bacc
from concourse.bass_utils import run_bass_kernel_spmd

F32 = mybir.dt.float32
BF16 = mybir.dt.bfloat16
AF = mybir.ActivationFunctionType
ALU = mybir.AluOpType

N, DV, M, DE, E = 2048, 32, 4096, 16, 65536
D, WD, L, H = 256, 64, 2, 4
NCORES = 8
MASK_C = 30.0


def _sab_layer(nc, pools, items, ident, negc):
    """One SAB layer on a group of stage-interleaved 256-row tiles.

    items: list of (Xt, C) where Xt is a [feat, rows] pair of [128,256]
    bf16 tiles and C this tile's layer consts.  Returns the new Xt pairs.
    """
    sb, ps = pools
    n = len(items)

    # --- Q/K projections (feature-major) ---
    psQ, psK = [], []
    for i, (Xt, C) in enumerate(items):
        W = C[0]
        q = ps.tile([128, 512], F32, tag="bank", name="psQ")
        k = ps.tile([128, 512], F32, tag="bank", name="psK")
        for fb in range(2):
            for kb in range(2):
                nc.tensor.matmul(q[:, fb * 256:(fb + 1) * 256],
                                 W["q"][kb][:, fb * 128:(fb + 1) * 128],
                                 Xt[kb][:], start=(kb == 0), stop=(kb == 1))
                nc.tensor.matmul(k[:, fb * 256:(fb + 1) * 256],
                                 W["k"][kb][:, fb * 128:(fb + 1) * 128],
                                 Xt[kb][:], start=(kb == 0), stop=(kb == 1))
        psQ.append(q)
        psK.append(k)
    Qt = [[sb.tile([128, 256], BF16, tag="Qt", name="Qt", bufs=4 * n)
           for _ in range(2)] for _ in range(n)]
    Kt = [[sb.tile([128, 256], BF16, tag="Kt", name="Kt", bufs=4 * n)
           for _ in range(2)] for _ in range(n)]
    for i, (Xt, C) in enumerate(items):
        Bq = C[1]
        for fb in range(2):
            # bq folded into the evacuation; bk cancels in softmax -> copy.
            nc.scalar.activation(Qt[i][fb][:], psQ[i][:, fb * 256:(fb + 1) * 256],
                                 AF.Identity, bias=Bq[:, fb:fb + 1])
            nc.gpsimd.tensor_copy(Kt[i][fb][:], psK[i][:, fb * 256:(fb + 1) * 256])

    # --- V projection (row-major, 65-stride with ones column) ---
    psV = []
    for i, (Xt, C) in enumerate(items):
        W = C[0]
        v = ps.tile([128, 2, 4, 64], F32, tag="bank", name="psV")
        for rb in range(2):
            for kb in range(2):
                nc.tensor.matmul(v[:, rb, :, :],
                                 Xt[kb][:, rb * 128:(rb + 1) * 128],
                                 W["v"][kb][:], start=(kb == 0), stop=(kb == 1))
        psV.append(v)
    V65 = [sb.tile([128, 2, 4, 65], BF16, tag="V65", name="V65", bufs=2 * n)
           for _ in range(n)]
    for i in range(n):
        nc.gpsimd.tensor_copy(V65[i][:, 0, :, 0:64], psV[i][:, 0, :, :])
        nc.scalar.copy(V65[i][:, 1, :, 0:64], psV[i][:, 1, :, :])
        nc.gpsimd.memset(V65[i][:, :, :, 64:65], 1.0)

    # --- attention per diagonal 128x128 block ---
    O_rm = [sb.tile([128, 2, 256], BF16, tag="Orm", name="O_rm", bufs=2 * n)
            for _ in range(n)]
    for b in range(2):
        psS, eS = [], []
        for i, (Xt, C) in enumerate(items):
            Gk, Gq4 = C[4], C[5]
            s = ps.tile([128, 512], F32, tag="bank", name="psS")
            nc.tensor.matmul(s[:], Gk[:], Gq4[:], start=True, stop=False)
            for h in range(H):
                p, off = h // 2, (h % 2) * 64
                bs = slice(b * 128, (b + 1) * 128)
                nc.tensor.matmul(s[:, h * 128:(h + 1) * 128],
                                 Kt[i][p][off:off + 64, bs],
                                 Qt[i][p][off:off + 64, bs],
                                 start=False, stop=True)
            psS.append(s)
        for i in range(n):
            e = sb.tile([128, 512], BF16, tag="eS", name="eS", bufs=2 * n)
            nc.scalar.activation(e[:], psS[i][:], AF.Exp, bias=negc[:],
                                 scale=1.0 / 16.0)
            eS.append(e)
        for i in range(n):
            pO = ps.tile([128, 4, 65], F32, tag="bank", name="pO")
            for h in range(H):
                nc.tensor.matmul(pO[:, h, :], eS[i][:, h * 128:(h + 1) * 128],
                                 V65[i][:, b, h, :], start=True, stop=True)
            rec = sb.tile([128, 4, 1], F32, tag="rec", name="rec", bufs=2 * n)
            nc.vector.reciprocal(rec[:], pO[:, :, 64:65])
            for h in range(H):
                eng = nc.vector if h < 3 else nc.gpsimd
                eng.tensor_scalar_mul(O_rm[i][:, b, h * 64:(h + 1) * 64],
                                      pO[:, h, 0:64], rec[:, h, 0:1])

    # --- transpose O back to feature-major, + bv + Qh ---
    Ot = [[sb.tile([128, 256], BF16, tag="Ot", name="Ot", bufs=4 * n)
           for _ in range(2)] for _ in range(n)]
    for i, (Xt, C) in enumerate(items):
        Bv = C[3]
        psOT = ps.tile([128, 2, 256], BF16, tag="bank", name="psOT")
        for b in range(2):
            for fb in range(2):
                nc.tensor.transpose(psOT[:, fb, b * 128:(b + 1) * 128],
                                    O_rm[i][:, b, fb * 128:(fb + 1) * 128],
                                    ident[:])
        for fb in range(2):
            nc.vector.scalar_tensor_tensor(Ot[i][fb][:], psOT[:, fb, :],
                                           Bv[:, fb:fb + 1], Qt[i][fb][:],
                                           ALU.add, ALU.add)

    # --- Z = O + relu(O @ Wo + bo) ---
    psR = []
    for i, (Xt, C) in enumerate(items):
        W = C[0]
        r = ps.tile([128, 512], F32, tag="bank", name="psR")
        for fb in range(2):
            for kb in range(2):
                nc.tensor.matmul(r[:, fb * 256:(fb + 1) * 256],
                                 W["o"][kb][:, fb * 128:(fb + 1) * 128],
                                 Ot[i][kb][:], start=(kb == 0), stop=(kb == 1))
        psR.append(r)
    out = []
    for i, (Xt, C) in enumerate(items):
        Bo = C[2]
        Rt = [sb.tile([128, 256], BF16, tag="Rt", name="Rt", bufs=2 * n)
              for _ in range(2)]
        nc.scalar.activation(Rt[0][:], psR[i][:, 0:256], AF.Relu,
                             bias=Bo[:, 0:1])
        nc.gpsimd.tensor_scalar(Rt[1][:], psR[i][:, 256:512], Bo[:, 1:2], 0.0,
                                ALU.add, ALU.max)
        Zt = [sb.tile([128, 256], BF16, tag="Zt", name="Zt", bufs=4 * n)
              for _ in range(2)]
        for fb in range(2):
            nc.vector.tensor_add(Zt[fb][:], Ot[i][fb][:], Rt[fb][:])
        out.append((Zt, C))
    return out


def _load_side_consts(nc, const, tag, W_d, Bq_d, Bo_d, Bv_d, Gk_d, Gq4_d, G):
    layers = []
    for l in range(L):
        Wl = {}
        for pi, p in enumerate(["q", "k", "v", "o"]):
            Wl[p] = []
            for kb in range(2):
                t = const.tile([128, 256], BF16, tag=f"{tag}W{l}{p}{kb}",
                               name=f"{tag}W{l}{p}{kb}")
                nc.sync.dma_start(t[:], W_d[l, pi, kb * 128:(kb + 1) * 128, :])
                Wl[p].append(t)
        bq = const.tile([128, 2], F32, tag=f"{tag}Bq{l}", name=f"{tag}Bq{l}")
        nc.sync.dma_start(bq[:], Bq_d[l])
        bo = const.tile([128, 2], F32, tag=f"{tag}Bo{l}", name=f"{tag}Bo{l}")
        nc.sync.dma_start(bo[:], Bo_d[l])
        bv = const.tile([128, 2], F32, tag=f"{tag}Bv{l}", name=f"{tag}Bv{l}")
        nc.sync.dma_start(bv[:], Bv_d[l])
        layers.append((Wl, bq, bo, bv))
    Gk = const.tile([G, 128], BF16, tag=f"{tag}Gk", name=f"{tag}Gk")
    nc.sync.dma_start(Gk[:], Gk_d)
    Gq4 = const.tile([G, 512], BF16, tag=f"{tag}Gq4", name=f"{tag}Gq4")
    nc.sync.dma_start(Gq4[:], Gq4_d)
    return [(Wl, bq, bo, bv, Gk, Gq4) for (Wl, bq, bo, bv) in layers]


def build_program(R):
    """Build the per-core SPMD program; R = rows per core (multiple of 256)."""
    NT = R // 256
    nc = bacc.Bacc("TRN2", target_bir_lowering=False, debug=False)

    def din(name, shape, dt=BF16):
        return nc.dram_tensor(name, shape, dt, kind="ExternalInput").ap()

    xvt_d = din("xvt", [D, R])
    wvt_d = din("wvt", [WD, R])
    xet_d = din("xet", [D, R])
    wet_d = din("wet", [WD, R])
    x0t_d = din("x0t", [D, R])
    peW_v_d = din("peW_v", [WD, D])
    peW_e_d = din("peW_e", [WD, D])
    peb_v_d = din("peb_v", [128, 2], F32)
    peb_e_d = din("peb_e", [128, 2], F32)
    Wv_d = din("W_v", [L, 4, D, D])
    We_d = din("W_e", [L, 4, D, D])
    Bq_v_d = din("Bq_v", [L, 128, 2], F32)
    Bq_e_d = din("Bq_e", [L, 128, 2], F32)
    Bo_v_d = din("Bo_v", [L, 128, 2], F32)
    Bo_e_d = din("Bo_e", [L, 128, 2], F32)
    Bv_v_d = din("Bv_v", [L, 128, 2], F32)
    Bv_e_d = din("Bv_e", [L, 128, 2], F32)
    Wupd_d = din("W_upd", [4, D, D])
    updb_d = din("updb", [128, 2], F32)
    Gk_v_d = din("Gk_v", [4, 128])
    Gq4_v_d = din("Gq4_v", [4, 512])
    Gk_e_d = din("Gk_e", [8, 128])
    Gq4_e_d = din("Gq4_e", [8, 512])
    ident_d = din("ident", [128, 128])

    At_d = nc.dram_tensor("At", [D, R], BF16, kind="ExternalOutput").ap()
    P3t_d = nc.dram_tensor("P3t", [D, R], BF16, kind="ExternalOutput").ap()

    with tile.TileContext(nc) as tc, ExitStack() as es, \
            nc.allow_low_precision(reason="bf16 activations, fp32 PSUM accum"):
        const = es.enter_context(tc.tile_pool(name="const", bufs=1))
        sb = es.enter_context(tc.tile_pool(name="sb", bufs=4))
        inp = es.enter_context(tc.tile_pool(name="inp", bufs=6))
        outp = es.enter_context(tc.tile_pool(name="outp", bufs=6))
        ps = es.enter_context(tc.tile_pool(name="ps", bufs=8, space="PSUM"))
        pools = (sb, ps)

        negc = const.tile([128, 1], F32, tag="negc", name="negc")
        nc.vector.memset(negc[:], -MASK_C)
        ident = const.tile([128, 128], BF16, tag="ident", name="ident")
        nc.sync.dma_start(ident[:], ident_d)

        peW = {}
        peb = {}
        for s, peW_d, peb_d in (("v", peW_v_d, peb_v_d), ("e", peW_e_d, peb_e_d)):
            t = const.tile([WD, D], BF16, tag=f"peW_{s}", name=f"peW_{s}")
            nc.sync.dma_start(t[:], peW_d)
            peW[s] = t
            b = const.tile([128, 2], F32, tag=f"peb_{s}", name=f"peb_{s}")
            nc.sync.dma_start(b[:], peb_d)
            peb[s] = b

        side_consts = {
            "v": _load_side_consts(nc, const, "v", Wv_d, Bq_v_d, Bo_v_d, Bv_v_d,
                                   Gk_v_d, Gq4_v_d, 4),
            "e": _load_side_consts(nc, const, "e", We_d, Bq_e_d, Bo_e_d, Bv_e_d,
                                   Gk_e_d, Gq4_e_d, 8),
        }

        Wupd = []
        for j in range(4):
            Wupd.append([])
            for kb in range(2):
                t = const.tile([128, 256], BF16, tag=f"Wupd{j}{kb}",
                               name=f"Wupd{j}{kb}")
                nc.sync.dma_start(t[:], Wupd_d[j, kb * 128:(kb + 1) * 128, :])
                Wupd[j].append(t)
        updb = const.tile([128, 2], F32, tag="updb", name="updb")
        nc.sync.dma_start(updb[:], updb_d)

        sides = (("v", xvt_d, wvt_d, At_d), ("e", xet_d, wet_d, P3t_d))
        for t in range(NT):
            cs = slice(t * 256, (t + 1) * 256)
            # --- loads + mailbox prep, interleaved across both sides ---
            xts, wts = {}, {}
            for s, xt_d, wt_d, _ in sides:
                xts[s] = [inp.tile([128, 256], BF16, tag=f"xt{s}", name="xt")
                          for _ in range(2)]
                for fb in range(2):
                    nc.sync.dma_start(xts[s][fb][:],
                                      xt_d[fb * 128:(fb + 1) * 128, cs])
                wts[s] = inp.tile([WD, 256], BF16, tag=f"wt{s}", name="wt")
                nc.sync.dma_start(wts[s][:], wt_d[:, cs])
            x0 = [inp.tile([128, 256], BF16, tag="x0", name="x0")
                  for _ in range(2)]
            for fb in range(2):
                nc.sync.dma_start(x0[fb][:], x0t_d[fb * 128:(fb + 1) * 128, cs])

            psP, Xts = {}, {}
            for s, _, _, _ in sides:
                psP[s] = ps.tile([128, 512], F32, tag="bank", name="psP")
                for fb in range(2):
                    nc.tensor.matmul(psP[s][:, fb * 256:(fb + 1) * 256],
                                     peW[s][:, fb * 128:(fb + 1) * 128],
                                     wts[s][:], start=True, stop=True)
            for s, _, _, _ in sides:
                Xts[s] = [sb.tile([128, 256], BF16, tag="Xt", name="Xt", bufs=8)
                          for _ in range(2)]
                for fb in range(2):
                    nc.vector.scalar_tensor_tensor(
                        Xts[s][fb][:], psP[s][:, fb * 256:(fb + 1) * 256],
                        peb[s][:, fb:fb + 1], xts[s][fb][:], ALU.add, ALU.add)

            # --- 2 SAB layers, stage-interleaved across sides ---
            items = [(Xts["v"], side_consts["v"][0]),
                     (Xts["e"], side_consts["e"][0])]
            items = _sab_layer(nc, pools, items, ident, negc)
            items = [(items[0][0], side_consts["v"][1]),
                     (items[1][0], side_consts["e"][1])]
            items = _sab_layer(nc, pools, items, ident, negc)
            Zv, Ze = items[0][0], items[1][0]

            # --- update linear, feature-major: out[of, rows] ---
            psA = {}
            for s, Zt in (("v", Zv), ("e", Ze)):
                a = ps.tile([128, 512], F32, tag="bank", name="psA")
                srcs = (((xts["v"], 0), (Zt, 1), (x0, 3)) if s == "v"
                        else ((Zt, 2),))
                for of in range(2):
                    first = True
                    for src, j in srcs:
                        for kb in range(2):
                            nc.tensor.matmul(
                                a[:, of * 256:(of + 1) * 256],
                                Wupd[j][kb][:, of * 128:(of + 1) * 128],
                                src[kb][:], start=first,
                                stop=(src is srcs[-1][0] and kb == 1))
                            first = False
                psA[s] = a
            for s, _, _, out_d in sides:
                Ao = outp.tile([128, 512], BF16, tag=f"Ao{s}", name="Ao")
                for of in range(2):
                    if s == "v":
                        nc.gpsimd.tensor_scalar_add(
                            Ao[:, of * 256:(of + 1) * 256],
                            psA[s][:, of * 256:(of + 1) * 256],
                            updb[:, of:of + 1])
                    else:
                        nc.scalar.copy(Ao[:, of * 256:(of + 1) * 256],
                                       psA[s][:, of * 256:(of + 1) * 256])
                    nc.sync.dma_start(out_d[of * 128:(of + 1) * 128, cs],
                                      Ao[:, of * 256:(of + 1) * 256])

    nc.compile()
    return nc


def _make_group_consts(n_group):
    """Gk [G,128] (bf16, 16C on own group) and Gq4 [G,512] (1 on own group,
    tiled for 4 heads)."""
    G = 128 // n_group
    Gk = np.zeros((G, 128), np.float32)
    Gq = np.zeros((G, 128), np.float32)
    for g in range(G):
        Gk[g, g * n_group:(g + 1) * n_group] = 16.0 * MASK_C
        Gq[g, g * n_group:(g + 1) * n_group] = 1.0
    Gq4 = np.tile(Gq, (1, 4))
    return Gk, Gq4


def _pack_cols(b):
    """[L, D] -> [L, 128, 2] per-partition bias columns per feature chunk."""
    out = np.zeros((L, 128, 2), np.float32)
    for l in range(L):
        for fb in range(2):
            out[l, :, fb] = b[l, fb * 128:(fb + 1) * 128]
    return out


_PROGRAM_CACHE = {}


def _get_program(R):
    if R not in _PROGRAM_CACHE:
        _PROGRAM_CACHE[R] = build_program(R)
    return _PROGRAM_CACHE[R]


def kernel(co_feat_in, co_feat_con, co_feat_0, weight_in, weight_con,
           pe_v_W, pe_v_b, pe_e_W, pe_e_b,
           Wq_v, bq_v, Wk_v, bk_v, Wv_v, bv_v, Wo_v, bo_v,
           Wq_e, bq_e, Wk_e, bk_e, Wv_e, bv_e, Wo_e, bo_e,
           upd_W, upd_b, perm):
    bf = np.dtype(mybir.dt.np(BF16))
    f = lambda x: np.asarray(x, np.float32)
    perm = np.asarray(perm)

    R = E // NCORES
    nc = _get_program(R)

    Gk_v, Gq4_v = _make_group_consts(DV)
    Gk_e, Gq4_e = _make_group_consts(DE)

    def col2(b):
        return np.stack([f(b)[0:128], f(b)[128:256]], axis=1).copy()

    shared = {
        "peW_v": f(pe_v_W).astype(bf), "peW_e": f(pe_e_W).astype(bf),
        "peb_v": col2(pe_v_b), "peb_e": col2(pe_e_b),
        "W_v": np.stack([f(Wq_v), f(Wk_v), f(Wv_v), f(Wo_v)],
                        axis=1).astype(bf).copy(),
        "W_e": np.stack([f(Wq_e), f(Wk_e), f(Wv_e), f(Wo_e)],
                        axis=1).astype(bf).copy(),
        "Bq_v": _pack_cols(f(bq_v)), "Bq_e": _pack_cols(f(bq_e)),
        "Bo_v": _pack_cols(f(bo_v)), "Bo_e": _pack_cols(f(bo_e)),
        "Bv_v": _pack_cols(f(bv_v)), "Bv_e": _pack_cols(f(bv_e)),
        "W_upd": np.ascontiguousarray(f(upd_W).reshape(4, D, D)).astype(bf),
        "updb": col2(upd_b),
        "Gk_v": Gk_v.astype(bf), "Gq4_v": Gq4_v.astype(bf),
        "Gk_e": Gk_e.astype(bf), "Gq4_e": Gq4_e.astype(bf),
        "ident": np.eye(128, dtype=np.float32).astype(bf),
    }

    in_maps = []
    for c in range(NCORES):
        rs = slice(c * R, (c + 1) * R)
        m = dict(shared)
        m["xvt"] = np.asarray(co_feat_in)[rs].T.astype(bf)
        m["wvt"] = np.asarray(weight_in)[rs].T.astype(bf)
        m["xet"] = np.asarray(co_feat_con)[rs].T.astype(bf)
        m["wet"] = np.asarray(weight_con)[rs].T.astype(bf)
        m["x0t"] = np.asarray(co_feat_0)[rs].T.astype(bf)
        in_maps.append(m)

    res = run_bass_kernel_spmd(nc, in_maps, core_ids=list(range(NCORES)))
    A = np.concatenate(
        [res.results[c]["At"].T.astype(np.float32) for c in range(NCORES)], axis=0)
    P3 = np.concatenate(
        [res.results[c]["P3t"].T.astype(np.float32) for c in range(NCORES)], axis=0)

    inv_perm = np.argsort(perm)
    out_in = A + P3[inv_perm]
    return np.stack([out_in, out_in[perm]]).astype(np.float32)


# revision 10
# speedup vs baseline: 2.0240x; 1.1961x over previous
"""CoNHD GD-layer Trainium2 kernel (8-core SPMD, Bass/Tile), v5.

Math (see the reference): two independent set-attention stacks over
fixed-size mailbox groups (v-side: N=2048 nodes x DV=32, e-side: M=4096
hyperedges x DE=16), followed by a 4*D -> D update linear applied in two
eid orders.

Device strategy (engine-balanced, block-diagonal attention, 2-way
software pipelining):
  - Shard rows (E=65536) across 8 cores; group attention never crosses
    the per-core boundary.  Activations are bf16 on chip; PSUM accum fp32.
  - A v-side tile and an e-side tile are processed as a stage-interleaved
    pair: their dependency chains are independent, so each engine's
    in-order queue always has work from the other tile while one tile
    waits on a cross-engine dependency.
  - Scores are computed only for the two diagonal 128x128 key x query
    blocks of each 256-row tile (groups of 32/16 never cross them), all
    4 heads sharing one [128,512] PSUM bank; the block-diagonal group
    mask is a rank-G accumulating matmul (exp(x/16 - C) removes 16C).
  - AV is computed transposed (queries on partitions); ap=1 ones-matmuls
    produce the softmax denominators in a dedicated small PSUM pool, so
    normalization is a cheap per-partition tensor_scalar.
  - Normalized O is transposed back to feature-major on the PE (identity
    matmul) and +Qh is a single fused add.
  - bk provably cancels in softmax and is dropped.  When all biases are
    zero (the spec fills them with zeros) the fast path fuses every
    PSUM evacuation into one [128,(2,256)] op; otherwise a general path
    adds per-chunk per-partition biases.
  - The Activation engine's queue carries only critical-path work (exp)
    plus post-score ops, so scores -> exp -> AV never queues behind bulk
    evacuations.  Evacuations are spread across DVE / GPSIMD.
  - The update linear runs feature-major (out [of, rows]); outputs land
    transposed in HBM ([128, 2, R] bf16) and the host re-assembles +
    combines the two eid orders:
      out_in  = A + P3[inv_perm],  out_con = out_in[perm].
  - Host-side layouts make every per-tile DMA a single [128,2,256]
    transfer; weights load as a handful of blob DMAs (the SP sequencer's
    ~650ns per-DMA issue cost would otherwise serialize).

kernel(**inputs) takes the full unsharded inputs and returns [2, E, D] f32.
"""
import sys

if "/opt/trn_rl_repo" not in sys.path:
    sys.path.insert(0, "/opt/trn_rl_repo")

from contextlib import ExitStack

import numpy as np

import concourse.mybir as mybir
import concourse.tile as tile
from concourse import bacc
from concourse.bass_utils import run_bass_kernel_spmd

F32 = mybir.dt.float32
BF16 = mybir.dt.bfloat16
AF = mybir.ActivationFunctionType
ALU = mybir.AluOpType

N, DV, M, DE, E = 2048, 32, 4096, 16, 65536
D, WD, L, H = 256, 64, 2, 4
NCORES = 8
MASK_C = 30.0
PS_B = 8  # 2KB PSUM slots (banks) in the main pool


def _sab_layer(nc, pools, items, zb, ident, negc, ones1):
    """One SAB layer on a group of stage-interleaved 256-row tiles.

    items: list of (Xt, C); Xt is a [128, 2, 256] bf16 tile (feature-major,
    fb chunks).  Returns the new (Xt, C) list.
    """
    sb, ps = pools
    n = len(items)

    # --- Q/K projections (feature-major) ---
    psQ, psK = [], []
    for i, (Xt, C) in enumerate(items):
        W = C["W"]
        q = ps.tile([128, 2, 256], F32, tag="bank", name="psQ", bufs=PS_B)
        k = ps.tile([128, 2, 256], F32, tag="bank", name="psK", bufs=PS_B)
        for fb in range(2):
            for kb in range(2):
                nc.tensor.matmul(q[:, fb, :], W("q", kb, fb),
                                 Xt[:, kb, :], start=(kb == 0), stop=(kb == 1))
                nc.tensor.matmul(k[:, fb, :], W("k", kb, fb),
                                 Xt[:, kb, :], start=(kb == 0), stop=(kb == 1))
        psQ.append(q)
        psK.append(k)
    Qt = [sb.tile([128, 2, 256], BF16, tag="Qt", name="Qt", bufs=3 * n)
          for _ in range(n)]
    Kt = [sb.tile([128, 2, 256], BF16, tag="Kt", name="Kt", bufs=3 * n)
          for _ in range(n)]
    for i, (Xt, C) in enumerate(items):
        if zb:
            nc.vector.tensor_copy(Qt[i][:], psQ[i][:])
        else:
            Bq = C["Bq"]
            for fb in range(2):
                nc.vector.tensor_scalar_add(Qt[i][:, fb, :], psQ[i][:, fb, :],
                                            Bq[:, fb:fb + 1])
        nc.gpsimd.tensor_copy(Kt[i][:], psK[i][:])

    # --- V projection (row-major) ---
    psV = []
    for i, (Xt, C) in enumerate(items):
        W = C["W"]
        v = ps.tile([128, 2, 4, 64], F32, tag="bank", name="psV", bufs=PS_B)
        for rb in range(2):
            for kb in range(2):
                nc.tensor.matmul(v[:, rb, :, :],
                                 Xt[:, kb, rb * 128:(rb + 1) * 128],
                                 W("v", kb, None), start=(kb == 0), stop=(kb == 1))
        psV.append(v)
    Vr = [sb.tile([128, 2, 4, 64], BF16, tag="Vr", name="Vr", bufs=2 * n)
          for _ in range(n)]
    for i in range(n):
        nc.gpsimd.tensor_copy(Vr[i][:], psV[i][:])

    # --- attention per diagonal 128x128 block ---
    O_rm = [sb.tile([128, 2, 256], BF16, tag="Orm", name="O_rm", bufs=2 * n)
            for _ in range(n)]
    for b in range(2):
        psS = []
        for i, (Xt, C) in enumerate(items):
            s = ps.tile([128, 512], F32, tag="bank", name="psS", bufs=PS_B)
            nc.tensor.matmul(s[:], C["Gk"][:], C["Gq4"][:], start=True, stop=False)
            for h in range(H):
                p, off = h // 2, (h % 2) * 64
                bs = slice(b * 128, (b + 1) * 128)
                nc.tensor.matmul(s[:, h * 128:(h + 1) * 128],
                                 Kt[i][off:off + 64, p, bs],
                                 Qt[i][off:off + 64, p, bs],
                                 start=False, stop=True)
            psS.append(s)
        eS = []
        for i in range(n):
            e = sb.tile([128, 512], BF16, tag="eS", name="eS", bufs=2 * n)
            nc.scalar.activation(e[:], psS[i][:], AF.Exp, bias=negc[:],
                                 scale=1.0 / 16.0)
            eS.append(e)
        for i in range(n):
            pO = ps.tile([128, 4, 65], F32, tag="bank", name="pO", bufs=PS_B)
            for h in range(H):
                nc.tensor.matmul(pO[:, h, 0:64], eS[i][:, h * 128:(h + 1) * 128],
                                 Vr[i][:, b, h, :], start=True, stop=True)
                nc.tensor.matmul(pO[:, h, 64:65],
                                 eS[i][:, h * 128:(h + 1) * 128],
                                 ones1[:], start=True, stop=True)
            rec = sb.tile([128, 4, 1], F32, tag="rec", name="rec", bufs=2 * n)
            nc.vector.reciprocal(rec[:], pO[:, :, 64:65])
            for h in range(H):
                eng = nc.vector if h < 2 else nc.gpsimd
                eng.tensor_scalar_mul(O_rm[i][:, b, h * 64:(h + 1) * 64],
                                      pO[:, h, 0:64], rec[:, h, 0:1])

    # --- transpose O back to feature-major, + Qh ---
    Ot = [sb.tile([128, 2, 256], BF16, tag="Ot", name="Ot", bufs=3 * n)
          for _ in range(n)]
    for i, (Xt, C) in enumerate(items):
        psOT = ps.tile([128, 2, 256], BF16, tag="bank", name="psOT", bufs=PS_B)
        for b in range(2):
            for fb in range(2):
                nc.tensor.transpose(psOT[:, fb, b * 128:(b + 1) * 128],
                                    O_rm[i][:, b, fb * 128:(fb + 1) * 128],
                                    ident[:])
        if zb:
            nc.vector.tensor_add(Ot[i][:], psOT[:], Qt[i][:])
        else:
            Bv = C["Bv"]
            for fb in range(2):
                nc.vector.scalar_tensor_tensor(
                    Ot[i][:, fb, :], psOT[:, fb, :],
                    Bv[:, fb:fb + 1], Qt[i][:, fb, :], ALU.add, ALU.add)

    # --- Z = O + relu(O @ Wo + bo) ---
    psR = []
    for i, (Xt, C) in enumerate(items):
        W = C["W"]
        r = ps.tile([128, 2, 256], F32, tag="bank", name="psR", bufs=PS_B)
        for fb in range(2):
            for kb in range(2):
                nc.tensor.matmul(r[:, fb, :], W("o", kb, fb),
                                 Ot[i][:, kb, :], start=(kb == 0), stop=(kb == 1))
        psR.append(r)
    out = []
    for i, (Xt, C) in enumerate(items):
        Rt = sb.tile([128, 2, 256], BF16, tag="Rt", name="Rt", bufs=2 * n)
        if zb:
            nc.scalar.activation(Rt[:], psR[i][:], AF.Relu)
        else:
            Bo = C["Bo"]
            for fb in range(2):
                nc.scalar.activation(Rt[:, fb, :], psR[i][:, fb, :],
                                     AF.Relu, bias=Bo[:, fb:fb + 1])
        Zt = sb.tile([128, 2, 256], BF16, tag="Zt", name="Zt", bufs=3 * n)
        nc.vector.tensor_add(Zt[:], Ot[i][:], Rt[:])
        out.append((Zt, C))
    return out


def build_program(R, zb):
    """Per-core SPMD program; R = rows per core; zb = all biases zero."""
    NT = R // 256
    nc = bacc.Bacc("TRN2", target_bir_lowering=False, debug=False)

    def din(name, shape, dt=BF16):
        return nc.dram_tensor(name, shape, dt, kind="ExternalInput").ap()

    xvt_d = din("xvt", [128, 2, R])
    wvt_d = din("wvt", [WD, R])
    xet_d = din("xet", [128, 2, R])
    wet_d = din("wet", [WD, R])
    x0t_d = din("x0t", [128, 2, R])
    peW_v_d = din("peW_v", [WD, D])
    peW_e_d = din("peW_e", [WD, D])
    # W blobs: [128, L, 4(pi), 2(kb), 256]
    Wv_d = din("W_v", [128, L, 4, 2, 256])
    We_d = din("W_e", [128, L, 4, 2, 256])
    Wu_d = din("W_upd", [128, 4, 2, 256])
    # bias blob [128, 32] f32: cols 0..3 Bq_v(l,fb), 4..7 Bo_v, 8..11 Bv_v,
    # 12..23 same for e, 24..25 peb_v, 26..27 peb_e, 28..29 updb
    bias_d = din("biases", [128, 32], F32)
    Gk_v_d = din("Gk_v", [4, 128])
    Gq4_v_d = din("Gq4_v", [4, 512])
    Gk_e_d = din("Gk_e", [8, 128])
    Gq4_e_d = din("Gq4_e", [8, 512])
    ident_d = din("ident", [128, 128])

    At_d = nc.dram_tensor("At", [128, 2, R], BF16, kind="ExternalOutput").ap()
    P3t_d = nc.dram_tensor("P3t", [128, 2, R], BF16, kind="ExternalOutput").ap()

    with tile.TileContext(nc) as tc, ExitStack() as es, \
            nc.allow_low_precision(reason="bf16 activations, fp32 PSUM accum"):
        const = es.enter_context(tc.tile_pool(name="const", bufs=1))
        sb = es.enter_context(tc.tile_pool(name="sb", bufs=4))
        inp = es.enter_context(tc.tile_pool(name="inp", bufs=6))
        outp = es.enter_context(tc.tile_pool(name="outp", bufs=6))
        ps = es.enter_context(tc.tile_pool(name="ps", bufs=1, space="PSUM"))
        pools = (sb, ps)

        negc = const.tile([128, 1], F32, tag="negc", name="negc")
        nc.vector.memset(negc[:], -MASK_C)
        ones1 = const.tile([128, 1], BF16, tag="ones1", name="ones1")
        nc.vector.memset(ones1[:], 1.0)
        ident = const.tile([128, 128], BF16, tag="ident", name="ident")
        nc.sync.dma_start(ident[:], ident_d)

        bias = const.tile([128, 32], F32, tag="bias", name="bias")
        nc.sync.dma_start(bias[:], bias_d)

        peW = {}
        for s, peW_d in (("v", peW_v_d), ("e", peW_e_d)):
            t = const.tile([WD, D], BF16, tag=f"peW_{s}", name=f"peW_{s}")
            nc.sync.dma_start(t[:], peW_d)
            peW[s] = t
        peb = {"v": bias[:, 24:26], "e": bias[:, 26:28]}
        updb = bias[:, 28:30]

        Wall, G = {}, {}
        for s, W_d in (("v", Wv_d), ("e", We_d)):
            t = const.tile([128, L, 4, 2, 256], BF16, tag=f"W_{s}", name=f"W_{s}")
            nc.sync.dma_start(t[:], W_d)
            Wall[s] = t
        for s, gk_d, gq_d, g in (("v", Gk_v_d, Gq4_v_d, 4), ("e", Gk_e_d, Gq4_e_d, 8)):
            gk = const.tile([g, 128], BF16, tag=f"Gk{s}", name=f"Gk{s}")
            nc.sync.dma_start(gk[:], gk_d)
            gq = const.tile([g, 512], BF16, tag=f"Gq{s}", name=f"Gq{s}")
            nc.sync.dma_start(gq[:], gq_d)
            G[s] = (gk, gq)
        Wu = const.tile([128, 4, 2, 256], BF16, tag="Wu", name="Wu")
        nc.sync.dma_start(Wu[:], Wu_d)

        PI = {"q": 0, "k": 1, "v": 2, "o": 3}

        def mkW(s, l):
            def W(p, kb, fb):
                if fb is None:
                    return Wall[s][:, l, PI[p], kb, :]
                return Wall[s][:, l, PI[p], kb, fb * 128:(fb + 1) * 128]
            return W

        side_consts = {}
        for s in ("v", "e"):
            off = 0 if s == "v" else 12
            side_consts[s] = [{
                "W": mkW(s, l),
                "Bq": bias[:, off + l * 2:off + l * 2 + 2],
                "Bo": bias[:, off + 4 + l * 2:off + 4 + l * 2 + 2],
                "Bv": bias[:, off + 8 + l * 2:off + 8 + l * 2 + 2],
                "Gk": G[s][0], "Gq4": G[s][1],
            } for l in range(L)]

        sides = (("v", xvt_d, wvt_d, At_d), ("e", xet_d, wet_d, P3t_d))
        for t in range(NT):
            cs = slice(t * 256, (t + 1) * 256)
            xts, wts = {}, {}
            for s, xt_d, wt_d, _ in sides:
                xts[s] = inp.tile([128, 2, 256], BF16, tag=f"xt{s}", name="xt")
                nc.sync.dma_start(xts[s][:], xt_d[:, :, cs])
                wts[s] = inp.tile([WD, 256], BF16, tag=f"wt{s}", name="wt")
                nc.sync.dma_start(wts[s][:], wt_d[:, cs])
            x0 = inp.tile([128, 2, 256], BF16, tag="x0", name="x0")
            nc.sync.dma_start(x0[:], x0t_d[:, :, cs])

            psP, Xts = {}, {}
            for s, _, _, _ in sides:
                psP[s] = ps.tile([128, 2, 256], F32, tag="bank", name="psP",
                                 bufs=PS_B)
                for fb in range(2):
                    nc.tensor.matmul(psP[s][:, fb, :],
                                     peW[s][:, fb * 128:(fb + 1) * 128],
                                     wts[s][:], start=True, stop=True)
            for s, _, _, _ in sides:
                Xts[s] = sb.tile([128, 2, 256], BF16, tag="Xt", name="Xt",
                                 bufs=6)
                if zb:
                    nc.vector.tensor_add(Xts[s][:], psP[s][:], xts[s][:])
                else:
                    for fb in range(2):
                        nc.vector.scalar_tensor_tensor(
                            Xts[s][:, fb, :], psP[s][:, fb, :],
                            peb[s][:, fb:fb + 1], xts[s][:, fb, :],
                            ALU.add, ALU.add)

            # --- 2 SAB layers, stage-interleaved across sides ---
            items = [(Xts["v"], side_consts["v"][0]),
                     (Xts["e"], side_consts["e"][0])]
            items = _sab_layer(nc, pools, items, zb, ident, negc, ones1)
            items = [(items[0][0], side_consts["v"][1]),
                     (items[1][0], side_consts["e"][1])]
            items = _sab_layer(nc, pools, items, zb, ident, negc, ones1)
            Zv, Ze = items[0][0], items[1][0]

            # --- update linear, feature-major: out[of, rows] ---
            psA = {}
            for s, Zt in (("v", Zv), ("e", Ze)):
                a = ps.tile([128, 2, 256], F32, tag="bank", name="psA",
                            bufs=PS_B)
                srcs = (((xts["v"], 0), (Zt, 1), (x0, 3)) if s == "v"
                        else ((Zt, 2),))
                for of in range(2):
                    first = True
                    for src, j in srcs:
                        for kb in range(2):
                            nc.tensor.matmul(
                                a[:, of, :],
                                Wu[:, j, kb, of * 128:(of + 1) * 128],
                                src[:, kb, :], start=first,
                                stop=(src is srcs[-1][0] and kb == 1))
                            first = False
                psA[s] = a
            for s, _, _, out_d in sides:
                Ao = outp.tile([128, 2, 256], BF16, tag=f"Ao{s}", name="Ao")
                if zb or s == "e":
                    nc.scalar.copy(Ao[:], psA[s][:])
                else:
                    for of in range(2):
                        nc.scalar.activation(Ao[:, of, :], psA[s][:, of, :],
                                             AF.Identity,
                                             bias=updb[:, of:of + 1])
                nc.sync.dma_start(out_d[:, :, cs], Ao[:])

    nc.compile()
    return nc


def _make_group_consts(n_group):
    """Gk [G,128] (bf16, 16C on own group) and Gq4 [G,512] (1 on own group,
    tiled for 4 heads)."""
    G = 128 // n_group
    Gk = np.zeros((G, 128), np.float32)
    Gq = np.zeros((G, 128), np.float32)
    for g in range(G):
        Gk[g, g * n_group:(g + 1) * n_group] = 16.0 * MASK_C
        Gq[g, g * n_group:(g + 1) * n_group] = 1.0
    Gq4 = np.tile(Gq, (1, 4))
    return Gk, Gq4


_PROGRAM_CACHE = {}


def _get_program(R, zb=True):
    key = (R, zb)
    if key not in _PROGRAM_CACHE:
        _PROGRAM_CACHE[key] = build_program(R, zb)
    return _PROGRAM_CACHE[key]


def kernel(co_feat_in, co_feat_con, co_feat_0, weight_in, weight_con,
           pe_v_W, pe_v_b, pe_e_W, pe_e_b,
           Wq_v, bq_v, Wk_v, bk_v, Wv_v, bv_v, Wo_v, bo_v,
           Wq_e, bq_e, Wk_e, bk_e, Wv_e, bv_e, Wo_e, bo_e,
           upd_W, upd_b, perm):
    bf = np.dtype(mybir.dt.np(BF16))
    f = lambda x: np.asarray(x, np.float32)
    perm = np.asarray(perm)

    R = E // NCORES
    zb = all(not np.any(f(b)) for b in
             (bq_v, bv_v, bo_v, bq_e, bv_e, bo_e, pe_v_b, pe_e_b, upd_b))
    nc = _get_program(R, zb)

    Gk_v, Gq4_v = _make_group_consts(DV)
    Gk_e, Gq4_e = _make_group_consts(DE)

    # bias blob [128, 32]
    bias = np.zeros((128, 32), np.float32)
    for off, (bq, bo, bv) in ((0, (bq_v, bo_v, bv_v)), (12, (bq_e, bo_e, bv_e))):
        for l in range(L):
            for fb in range(2):
                bias[:, off + l * 2 + fb] = f(bq)[l, fb * 128:(fb + 1) * 128]
                bias[:, off + 4 + l * 2 + fb] = f(bo)[l, fb * 128:(fb + 1) * 128]
                bias[:, off + 8 + l * 2 + fb] = f(bv)[l, fb * 128:(fb + 1) * 128]
    for fb in range(2):
        bias[:, 24 + fb] = f(pe_v_b)[fb * 128:(fb + 1) * 128]
        bias[:, 26 + fb] = f(pe_e_b)[fb * 128:(fb + 1) * 128]
        bias[:, 28 + fb] = f(upd_b)[fb * 128:(fb + 1) * 128]

    def wblob(Wq, Wk, Wv, Wo):
        # [L,4,D,D] -> [128, L, 4, 2, 256]
        Ws = np.stack([f(Wq), f(Wk), f(Wv), f(Wo)], axis=1)  # [L,4,D,D]
        return np.ascontiguousarray(
            Ws.reshape(L, 4, 2, 128, 256).transpose(3, 0, 1, 2, 4)).astype(bf)

    Wu = np.ascontiguousarray(
        f(upd_W).reshape(4, 2, 128, 256).transpose(2, 0, 1, 3)).astype(bf)

    def xlayout(x, rs):
        # [R, 256] slice -> [128, 2, R]
        a = np.asarray(x)[rs]
        return np.ascontiguousarray(
            a.reshape(a.shape[0], 2, 128).transpose(2, 1, 0)).astype(bf)

    shared = {
        "peW_v": f(pe_v_W).astype(bf), "peW_e": f(pe_e_W).astype(bf),
        "W_v": wblob(Wq_v, Wk_v, Wv_v, Wo_v),
        "W_e": wblob(Wq_e, Wk_e, Wv_e, Wo_e),
        "W_upd": Wu,
        "biases": bias,
        "Gk_v": Gk_v.astype(bf), "Gq4_v": Gq4_v.astype(bf),
        "Gk_e": Gk_e.astype(bf), "Gq4_e": Gq4_e.astype(bf),
        "ident": np.eye(128, dtype=np.float32).astype(bf),
    }

    in_maps = []
    for c in range(NCORES):
        rs = slice(c * R, (c + 1) * R)
        m = dict(shared)
        m["xvt"] = xlayout(co_feat_in, rs)
        m["xet"] = xlayout(co_feat_con, rs)
        m["x0t"] = xlayout(co_feat_0, rs)
        m["wvt"] = np.asarray(weight_in)[rs].T.astype(bf)
        m["wet"] = np.asarray(weight_con)[rs].T.astype(bf)
        in_maps.append(m)

    res = run_bass_kernel_spmd(nc, in_maps, core_ids=list(range(NCORES)))

    def unlayout(o):
        # [128, 2, R] -> [R, 256] f32
        return np.asarray(o).transpose(2, 1, 0).reshape(R, 256).astype(np.float32)

    A = np.concatenate([unlayout(res.results[c]["At"]) for c in range(NCORES)])
    P3 = np.concatenate([unlayout(res.results[c]["P3t"]) for c in range(NCORES)])

    inv_perm = np.argsort(perm)
    out_in = A + P3[inv_perm]
    return np.stack([out_in, out_in[perm]]).astype(np.float32)


# revision 11
# speedup vs baseline: 2.4204x; 1.1959x over previous
"""CoNHD GD-layer Trainium2 kernel (8-core SPMD, Bass/Tile), v6.

Math (see the reference): two independent set-attention stacks over
fixed-size mailbox groups (v-side: N=2048 nodes x DV=32, e-side: M=4096
hyperedges x DE=16), followed by a 4*D -> D update linear applied in two
eid orders.

Device strategy (engine-balanced block-diagonal attention, 3-way
software pipelining):
  - Shard rows (E=65536) across 8 cores; group attention never crosses
    the per-core boundary.  Activations are bf16 on chip; PSUM accum fp32.
  - Tiles from both sides are processed in stage-interleaved groups of 3:
    independent dependency chains keep every engine's in-order queue fed
    while any one tile waits on a cross-engine dependency.
  - Scores are computed only for the two diagonal 128x128 key x query
    blocks of each 256-row tile (groups of 32/16 never cross them), all
    4 heads sharing one [128,512] PSUM bank; the block-diagonal group
    mask is a rank-G accumulating matmul (exp(x/16 - C) removes 16C).
  - AV is computed transposed (queries on partitions); ap=1 ones-matmuls
    append the softmax denominators as a 65th pO column, so normalization
    is one broadcast tensor_tensor per block (queries are partitions).
  - Normalized O is transposed back to feature-major on the PE (identity
    matmul) and +Qh is a single fused add.
  - bk provably cancels in softmax and is dropped.  When all biases are
    zero (the spec fills them with zeros) the fast path fuses every
    PSUM evacuation into one [128,(2,256)] op; otherwise a general path
    adds per-chunk per-partition biases.
  - The Activation engine's queue carries only critical-path work (exp,
    split per head-pair so AV starts early) plus post-score ops.
    Evacuations are spread across DVE / GPSIMD.
  - The update linear runs feature-major (out [of, rows]); outputs land
    transposed in HBM ([128, 2, R] bf16) and the host re-assembles +
    combines the two eid orders:
      out_in  = A + P3[inv_perm],  out_con = out_in[perm].
  - Host-side layouts make every per-tile DMA a single [128,2,256]
    transfer; weights load as a handful of blob DMAs (the SP sequencer's
    ~650ns per-DMA issue cost would otherwise serialize).

kernel(**inputs) takes the full unsharded inputs and returns [2, E, D] f32.
"""
import sys

if "/opt/trn_rl_repo" not in sys.path:
    sys.path.insert(0, "/opt/trn_rl_repo")

from contextlib import ExitStack

import numpy as np

import concourse.mybir as mybir
import concourse.tile as tile
from concourse import bacc
from concourse.bass_utils import run_bass_kernel_spmd

F32 = mybir.dt.float32
BF16 = mybir.dt.bfloat16
AF = mybir.ActivationFunctionType
ALU = mybir.AluOpType

N, DV, M, DE, E = 2048, 32, 4096, 16, 65536
D, WD, L, H = 256, 64, 2, 4
NCORES = 8
MASK_C = 30.0
PS_B = 8   # 2KB PSUM slots (banks) in the main pool
GRP = 3    # tiles processed stage-interleaved
SBUF_B = 3 * GRP


def _sab_layer(nc, pools, items, zb, ident, negc, ones1):
    """One SAB layer on a group of stage-interleaved 256-row tiles.

    items: list of (Xt, C); Xt is a [128, 2, 256] bf16 tile (feature-major,
    fb chunks).  Returns the new (Xt, C) list.
    """
    sb, ps = pools
    n = len(items)

    # --- Q/K projections (feature-major) ---
    psQ, psK = [], []
    for i, (Xt, C) in enumerate(items):
        W = C["W"]
        q = ps.tile([128, 2, 256], F32, tag="bank", name="psQ", bufs=PS_B)
        k = ps.tile([128, 2, 256], F32, tag="bank", name="psK", bufs=PS_B)
        for fb in range(2):
            for kb in range(2):
                nc.tensor.matmul(q[:, fb, :], W("q", kb, fb),
                                 Xt[:, kb, :], start=(kb == 0), stop=(kb == 1))
                nc.tensor.matmul(k[:, fb, :], W("k", kb, fb),
                                 Xt[:, kb, :], start=(kb == 0), stop=(kb == 1))
        psQ.append(q)
        psK.append(k)
    Qt = [sb.tile([128, 2, 256], BF16, tag="Qt", name="Qt", bufs=SBUF_B)
          for _ in range(n)]
    Kt = [sb.tile([128, 2, 256], BF16, tag="Kt", name="Kt", bufs=SBUF_B)
          for _ in range(n)]
    for i, (Xt, C) in enumerate(items):
        if zb:
            nc.vector.tensor_copy(Qt[i][:], psQ[i][:])
        else:
            Bq = C["Bq"]
            for fb in range(2):
                nc.vector.tensor_scalar_add(Qt[i][:, fb, :], psQ[i][:, fb, :],
                                            Bq[:, fb:fb + 1])
        nc.gpsimd.tensor_copy(Kt[i][:], psK[i][:])

    # --- V projection (row-major) ---
    psV = []
    for i, (Xt, C) in enumerate(items):
        W = C["W"]
        v = ps.tile([128, 2, 4, 64], F32, tag="bank", name="psV", bufs=PS_B)
        for rb in range(2):
            for kb in range(2):
                nc.tensor.matmul(v[:, rb, :, :],
                                 Xt[:, kb, rb * 128:(rb + 1) * 128],
                                 W("v", kb, None), start=(kb == 0), stop=(kb == 1))
        psV.append(v)
    Vr = [sb.tile([128, 2, 4, 64], BF16, tag="Vr", name="Vr", bufs=2 * GRP)
          for _ in range(n)]
    for i in range(n):
        nc.gpsimd.tensor_copy(Vr[i][:], psV[i][:])

    # --- attention per diagonal 128x128 block ---
    O_rm = [sb.tile([128, 2, 4, 64], BF16, tag="Orm", name="O_rm",
                    bufs=2 * GRP) for _ in range(n)]
    for b in range(2):
        psS = []
        for i, (Xt, C) in enumerate(items):
            s = ps.tile([128, 512], F32, tag="bank", name="psS", bufs=PS_B)
            nc.tensor.matmul(s[:], C["Gk"][:], C["Gq4"][:], start=True, stop=False)
            for h in range(H):
                p, off = h // 2, (h % 2) * 64
                bs = slice(b * 128, (b + 1) * 128)
                nc.tensor.matmul(s[:, h * 128:(h + 1) * 128],
                                 Kt[i][off:off + 64, p, bs],
                                 Qt[i][off:off + 64, p, bs],
                                 start=False, stop=True)
            psS.append(s)
        eS = []
        for i in range(n):
            e = sb.tile([128, 512], BF16, tag="eS", name="eS", bufs=2 * GRP)
            # split per head-pair so AV can start after the first half
            for hp in range(2):
                nc.scalar.activation(e[:, hp * 256:(hp + 1) * 256],
                                     psS[i][:, hp * 256:(hp + 1) * 256],
                                     AF.Exp, bias=negc[:], scale=1.0 / 16.0)
            eS.append(e)
        for i in range(n):
            pO = ps.tile([128, 4, 65], F32, tag="bank", name="pO", bufs=PS_B)
            for h in range(H):
                nc.tensor.matmul(pO[:, h, 0:64], eS[i][:, h * 128:(h + 1) * 128],
                                 Vr[i][:, b, h, :], start=True, stop=True)
                nc.tensor.matmul(pO[:, h, 64:65],
                                 eS[i][:, h * 128:(h + 1) * 128],
                                 ones1[:], start=True, stop=True)
            rec = sb.tile([128, 4, 1], F32, tag="rec", name="rec", bufs=2 * GRP)
            nc.vector.reciprocal(rec[:], pO[:, :, 64:65])
            eng = nc.vector if (b + i) % 2 == 0 else nc.gpsimd
            eng.tensor_tensor(O_rm[i][:, b, :, :], pO[:, :, 0:64],
                              rec[:].broadcast_to([128, 4, 64]), ALU.mult)

    # --- transpose O back to feature-major, + Qh ---
    Ot = [sb.tile([128, 2, 256], BF16, tag="Ot", name="Ot", bufs=SBUF_B)
          for _ in range(n)]
    for i, (Xt, C) in enumerate(items):
        psOT = ps.tile([128, 2, 256], BF16, tag="bank", name="psOT", bufs=PS_B)
        for b in range(2):
            for fb in range(2):
                nc.tensor.transpose(psOT[:, fb, b * 128:(b + 1) * 128],
                                    O_rm[i][:, b, fb * 2:fb * 2 + 2, :],
                                    ident[:])
        if zb:
            nc.vector.tensor_add(Ot[i][:], psOT[:], Qt[i][:])
        else:
            Bv = C["Bv"]
            for fb in range(2):
                nc.vector.scalar_tensor_tensor(
                    Ot[i][:, fb, :], psOT[:, fb, :],
                    Bv[:, fb:fb + 1], Qt[i][:, fb, :], ALU.add, ALU.add)

    # --- Z = O + relu(O @ Wo + bo) ---
    psR = []
    for i, (Xt, C) in enumerate(items):
        W = C["W"]
        r = ps.tile([128, 2, 256], F32, tag="bank", name="psR", bufs=PS_B)
        for fb in range(2):
            for kb in range(2):
                nc.tensor.matmul(r[:, fb, :], W("o", kb, fb),
                                 Ot[i][:, kb, :], start=(kb == 0), stop=(kb == 1))
        psR.append(r)
    out = []
    for i, (Xt, C) in enumerate(items):
        Rt = sb.tile([128, 2, 256], BF16, tag="Rt", name="Rt", bufs=2 * GRP)
        if zb:
            nc.scalar.activation(Rt[:], psR[i][:], AF.Relu)
        else:
            Bo = C["Bo"]
            for fb in range(2):
                nc.scalar.activation(Rt[:, fb, :], psR[i][:, fb, :],
                                     AF.Relu, bias=Bo[:, fb:fb + 1])
        Zt = sb.tile([128, 2, 256], BF16, tag="Zt", name="Zt", bufs=SBUF_B)
        nc.vector.tensor_add(Zt[:], Ot[i][:], Rt[:])
        out.append((Zt, C))
    return out


def build_program(R, zb):
    """Per-core SPMD program; R = rows per core; zb = all biases zero."""
    NT = R // 256
    nc = bacc.Bacc("TRN2", target_bir_lowering=False, debug=False)

    def din(name, shape, dt=BF16):
        return nc.dram_tensor(name, shape, dt, kind="ExternalInput").ap()

    xvt_d = din("xvt", [128, 2, R])
    wvt_d = din("wvt", [WD, R])
    xet_d = din("xet", [128, 2, R])
    wet_d = din("wet", [WD, R])
    x0t_d = din("x0t", [128, 2, R])
    peW_v_d = din("peW_v", [WD, D])
    peW_e_d = din("peW_e", [WD, D])
    Wv_d = din("W_v", [128, L, 4, 2, 256])
    We_d = din("W_e", [128, L, 4, 2, 256])
    Wu_d = din("W_upd", [128, 4, 2, 256])
    # bias blob [128, 32] f32: cols 0..3 Bq_v(l,fb), 4..7 Bo_v, 8..11 Bv_v,
    # 12..23 same for e, 24..25 peb_v, 26..27 peb_e, 28..29 updb
    bias_d = din("biases", [128, 32], F32)
    Gk_v_d = din("Gk_v", [4, 128])
    Gq4_v_d = din("Gq4_v", [4, 512])
    Gk_e_d = din("Gk_e", [8, 128])
    Gq4_e_d = din("Gq4_e", [8, 512])
    ident_d = din("ident", [128, 128])

    At_d = nc.dram_tensor("At", [128, 2, R], BF16, kind="ExternalOutput").ap()
    P3t_d = nc.dram_tensor("P3t", [128, 2, R], BF16, kind="ExternalOutput").ap()

    with tile.TileContext(nc) as tc, ExitStack() as es, \
            nc.allow_low_precision(reason="bf16 activations, fp32 PSUM accum"):
        const = es.enter_context(tc.tile_pool(name="const", bufs=1))
        sb = es.enter_context(tc.tile_pool(name="sb", bufs=4))
        inp = es.enter_context(tc.tile_pool(name="inp", bufs=3 * GRP))
        outp = es.enter_context(tc.tile_pool(name="outp", bufs=2 * GRP))
        ps = es.enter_context(tc.tile_pool(name="ps", bufs=1, space="PSUM"))
        pools = (sb, ps)

        negc = const.tile([128, 1], F32, tag="negc", name="negc")
        nc.vector.memset(negc[:], -MASK_C)
        ones1 = const.tile([128, 1], BF16, tag="ones1", name="ones1")
        nc.vector.memset(ones1[:], 1.0)
        ident = const.tile([128, 128], BF16, tag="ident", name="ident")
        nc.sync.dma_start(ident[:], ident_d)

        bias = const.tile([128, 32], F32, tag="bias", name="bias")
        nc.sync.dma_start(bias[:], bias_d)

        peW = {}
        for s, peW_d in (("v", peW_v_d), ("e", peW_e_d)):
            t = const.tile([WD, D], BF16, tag=f"peW_{s}", name=f"peW_{s}")
            nc.sync.dma_start(t[:], peW_d)
            peW[s] = t
        peb = {"v": bias[:, 24:26], "e": bias[:, 26:28]}
        updb = bias[:, 28:30]

        Wall, G = {}, {}
        for s, W_d in (("v", Wv_d), ("e", We_d)):
            t = const.tile([128, L, 4, 2, 256], BF16, tag=f"W_{s}", name=f"W_{s}")
            nc.sync.dma_start(t[:], W_d)
            Wall[s] = t
        for s, gk_d, gq_d, g in (("v", Gk_v_d, Gq4_v_d, 4), ("e", Gk_e_d, Gq4_e_d, 8)):
            gk = const.tile([g, 128], BF16, tag=f"Gk{s}", name=f"Gk{s}")
            nc.sync.dma_start(gk[:], gk_d)
            gq = const.tile([g, 512], BF16, tag=f"Gq{s}", name=f"Gq{s}")
            nc.sync.dma_start(gq[:], gq_d)
            G[s] = (gk, gq)
        Wu = const.tile([128, 4, 2, 256], BF16, tag="Wu", name="Wu")
        nc.sync.dma_start(Wu[:], Wu_d)

        PI = {"q": 0, "k": 1, "v": 2, "o": 3}

        def mkW(s, l):
            def W(p, kb, fb):
                if fb is None:
                    return Wall[s][:, l, PI[p], kb, :]
                return Wall[s][:, l, PI[p], kb, fb * 128:(fb + 1) * 128]
            return W

        side_consts = {}
        for s in ("v", "e"):
            off = 0 if s == "v" else 12
            side_consts[s] = [{
                "W": mkW(s, l),
                "Bq": bias[:, off + l * 2:off + l * 2 + 2],
                "Bo": bias[:, off + 4 + l * 2:off + 4 + l * 2 + 2],
                "Bv": bias[:, off + 8 + l * 2:off + 8 + l * 2 + 2],
                "Gk": G[s][0], "Gq4": G[s][1],
            } for l in range(L)]

        side_io = {"v": (xvt_d, wvt_d, At_d), "e": (xet_d, wet_d, P3t_d)}
        tiles = [(s, t) for t in range(NT) for s in ("v", "e")]
        groups = [tiles[g:g + GRP] for g in range(0, len(tiles), GRP)]

        for grp in groups:
            # --- loads ---
            xts, wts, x0s = [], [], {}
            for s, t in grp:
                cs = slice(t * 256, (t + 1) * 256)
                xt_d, wt_d, _ = side_io[s]
                xt = inp.tile([128, 2, 256], BF16, tag="xt", name="xt")
                nc.sync.dma_start(xt[:], xt_d[:, :, cs])
                xts.append(xt)
                wt = inp.tile([WD, 256], BF16, tag="wt", name="wt")
                nc.sync.dma_start(wt[:], wt_d[:, cs])
                wts.append(wt)
                if s == "v" and t not in x0s:
                    x0 = inp.tile([128, 2, 256], BF16, tag="x0", name="x0",
                                  bufs=GRP)
                    nc.sync.dma_start(x0[:], x0t_d[:, :, cs])
                    x0s[t] = x0

            # --- mailbox prep: Xt = xt + peW^T wt (+ peb) ---
            psP, Xts = [], []
            for gi, (s, t) in enumerate(grp):
                p = ps.tile([128, 2, 256], F32, tag="bank", name="psP",
                            bufs=PS_B)
                for fb in range(2):
                    nc.tensor.matmul(p[:, fb, :],
                                     peW[s][:, fb * 128:(fb + 1) * 128],
                                     wts[gi][:], start=True, stop=True)
                psP.append(p)
            for gi, (s, t) in enumerate(grp):
                Xt = sb.tile([128, 2, 256], BF16, tag="Xt", name="Xt",
                             bufs=SBUF_B)
                if zb:
                    nc.vector.tensor_add(Xt[:], psP[gi][:], xts[gi][:])
                else:
                    for fb in range(2):
                        nc.vector.scalar_tensor_tensor(
                            Xt[:, fb, :], psP[gi][:, fb, :],
                            peb[s][:, fb:fb + 1], xts[gi][:, fb, :],
                            ALU.add, ALU.add)
                Xts.append(Xt)

            # --- 2 SAB layers, stage-interleaved across the group ---
            items = [(Xts[gi], side_consts[s][0]) for gi, (s, t) in enumerate(grp)]
            items = _sab_layer(nc, pools, items, zb, ident, negc, ones1)
            items = [(items[gi][0], side_consts[s][1])
                     for gi, (s, t) in enumerate(grp)]
            items = _sab_layer(nc, pools, items, zb, ident, negc, ones1)

            # --- update linear, feature-major: out[of, rows] ---
            psA = []
            for gi, (s, t) in enumerate(grp):
                Zt = items[gi][0]
                a = ps.tile([128, 2, 256], F32, tag="bank", name="psA",
                            bufs=PS_B)
                srcs = (((xts[gi], 0), (Zt, 1), (x0s[t], 3)) if s == "v"
                        else ((Zt, 2),))
                for of in range(2):
                    first = True
                    for src, j in srcs:
                        for kb in range(2):
                            nc.tensor.matmul(
                                a[:, of, :],
                                Wu[:, j, kb, of * 128:(of + 1) * 128],
                                src[:, kb, :], start=first,
                                stop=(src is srcs[-1][0] and kb == 1))
                            first = False
                psA.append(a)
            for gi, (s, t) in enumerate(grp):
                cs = slice(t * 256, (t + 1) * 256)
                out_d = side_io[s][2]
                Ao = outp.tile([128, 2, 256], BF16, tag="Ao", name="Ao")
                if zb or s == "e":
                    nc.scalar.copy(Ao[:], psA[gi][:])
                else:
                    for of in range(2):
                        nc.scalar.activation(Ao[:, of, :], psA[gi][:, of, :],
                                             AF.Identity,
                                             bias=updb[:, of:of + 1])
                nc.sync.dma_start(out_d[:, :, cs], Ao[:])

    nc.compile()
    return nc


def _make_group_consts(n_group):
    """Gk [G,128] (bf16, 16C on own group) and Gq4 [G,512] (1 on own group,
    tiled for 4 heads)."""
    G = 128 // n_group
    Gk = np.zeros((G, 128), np.float32)
    Gq = np.zeros((G, 128), np.float32)
    for g in range(G):
        Gk[g, g * n_group:(g + 1) * n_group] = 16.0 * MASK_C
        Gq[g, g * n_group:(g + 1) * n_group] = 1.0
    Gq4 = np.tile(Gq, (1, 4))
    return Gk, Gq4


_PROGRAM_CACHE = {}


def _get_program(R, zb=True):
    key = (R, zb)
    if key not in _PROGRAM_CACHE:
        _PROGRAM_CACHE[key] = build_program(R, zb)
    return _PROGRAM_CACHE[key]


def kernel(co_feat_in, co_feat_con, co_feat_0, weight_in, weight_con,
           pe_v_W, pe_v_b, pe_e_W, pe_e_b,
           Wq_v, bq_v, Wk_v, bk_v, Wv_v, bv_v, Wo_v, bo_v,
           Wq_e, bq_e, Wk_e, bk_e, Wv_e, bv_e, Wo_e, bo_e,
           upd_W, upd_b, perm):
    bf = np.dtype(mybir.dt.np(BF16))
    f = lambda x: np.asarray(x, np.float32)
    perm = np.asarray(perm)

    R = E // NCORES
    zb = all(not np.any(f(b)) for b in
             (bq_v, bv_v, bo_v, bq_e, bv_e, bo_e, pe_v_b, pe_e_b, upd_b))
    nc = _get_program(R, zb)

    Gk_v, Gq4_v = _make_group_consts(DV)
    Gk_e, Gq4_e = _make_group_consts(DE)

    # bias blob [128, 32]
    bias = np.zeros((128, 32), np.float32)
    for off, (bq, bo, bv) in ((0, (bq_v, bo_v, bv_v)), (12, (bq_e, bo_e, bv_e))):
        for l in range(L):
            for fb in range(2):
                bias[:, off + l * 2 + fb] = f(bq)[l, fb * 128:(fb + 1) * 128]
                bias[:, off + 4 + l * 2 + fb] = f(bo)[l, fb * 128:(fb + 1) * 128]
                bias[:, off + 8 + l * 2 + fb] = f(bv)[l, fb * 128:(fb + 1) * 128]
    for fb in range(2):
        bias[:, 24 + fb] = f(pe_v_b)[fb * 128:(fb + 1) * 128]
        bias[:, 26 + fb] = f(pe_e_b)[fb * 128:(fb + 1) * 128]
        bias[:, 28 + fb] = f(upd_b)[fb * 128:(fb + 1) * 128]

    def wblob(Wq, Wk, Wv, Wo):
        # [L,4,D,D] -> [128, L, 4, 2, 256]
        Ws = np.stack([f(Wq), f(Wk), f(Wv), f(Wo)], axis=1)  # [L,4,D,D]
        return np.ascontiguousarray(
            Ws.reshape(L, 4, 2, 128, 256).transpose(3, 0, 1, 2, 4)).astype(bf)

    Wu = np.ascontiguousarray(
        f(upd_W).reshape(4, 2, 128, 256).transpose(2, 0, 1, 3)).astype(bf)

    def xlayout(x, rs):
        # [R, 256] slice -> [128, 2, R]
        a = np.asarray(x)[rs]
        return np.ascontiguousarray(
            a.reshape(a.shape[0], 2, 128).transpose(2, 1, 0)).astype(bf)

    shared = {
        "peW_v": f(pe_v_W).astype(bf), "peW_e": f(pe_e_W).astype(bf),
        "W_v": wblob(Wq_v, Wk_v, Wv_v, Wo_v),
        "W_e": wblob(Wq_e, Wk_e, Wv_e, Wo_e),
        "W_upd": Wu,
        "biases": bias,
        "Gk_v": Gk_v.astype(bf), "Gq4_v": Gq4_v.astype(bf),
        "Gk_e": Gk_e.astype(bf), "Gq4_e": Gq4_e.astype(bf),
        "ident": np.eye(128, dtype=np.float32).astype(bf),
    }

    in_maps = []
    for c in range(NCORES):
        rs = slice(c * R, (c + 1) * R)
        m = dict(shared)
        m["xvt"] = xlayout(co_feat_in, rs)
        m["xet"] = xlayout(co_feat_con, rs)
        m["x0t"] = xlayout(co_feat_0, rs)
        m["wvt"] = np.asarray(weight_in)[rs].T.astype(bf)
        m["wet"] = np.asarray(weight_con)[rs].T.astype(bf)
        in_maps.append(m)

    res = run_bass_kernel_spmd(nc, in_maps, core_ids=list(range(NCORES)))

    def unlayout(o):
        # [128, 2, R] -> [R, 256] f32
        return np.asarray(o).transpose(2, 1, 0).reshape(R, 256).astype(np.float32)

    A = np.concatenate([unlayout(res.results[c]["At"]) for c in range(NCORES)])
    P3 = np.concatenate([unlayout(res.results[c]["P3t"]) for c in range(NCORES)])

    inv_perm = np.argsort(perm)
    out_in = A + P3[inv_perm]
    return np.stack([out_in, out_in[perm]]).astype(np.float32)
